# revision 53
# baseline (speedup 1.0000x reference)
"""Trainium2 Bass kernel: Conformer block (B=8, N=512, DIM=512, H=8, DH=64, FF=2048, CIN=1024, K=31).

Sharding: pure data-parallel over batch — each of the 8 NeuronCores processes one
batch item with the full weight set (no collectives).

Layout: activations are kept FEATURE-major ([feature, time] = x.T) on chip so that
chained matmuls need no transposes (weights stay in natural [din, dout] layout as
the stationary operand).  LayerNorm reductions over features become ones-vector
matmuls on the PE; per-time-step affine factors are broadcast across partitions
with a GPSIMD partition_broadcast.

Relative-position attention uses the shift-gather trick: qr = q @ rel_emb.T is
bounced through an internal DRAM scratch and read back with a strided
(stride = row+1, step -1) access pattern so that rel[j, i] = qr[i, i-j+512]
lands directly as the transposed score tile.  Scores are computed transposed
(dots_T[j, i]) so softmax runs over the partition axis: exp on ACT, the
denominator via a ones-column fused into the attn@v matmul, and the final
normalization as a broadcasted multiply.

The causal depthwise conv runs on the PE as 31 PSUM-accumulated matmuls per
128-channel block against diagonal stationary matrices; the diagonals are
(re)written with a single strided DMA per block (dst step = row+1).

Matmuls use float32r (1 cycle/row for N>=256); the FFN second matmul and the
depthwise conv run in bf16.
"""

import sys

for _p in ("/opt/trn_rl_repo", "/root/.axon_site/_ro/trn_rl_repo"):
    if _p not in sys.path:
        sys.path.insert(0, _p)

import numpy as np

B, N, DIM, H, DH, MULT, EXP, KW, MAXP = 8, 512, 512, 8, 64, 4, 2, 31, 512
INNER = H * DH
FF = DIM * MULT
CIN = DIM * EXP
EPS = 1e-5
P = 128
DT = DIM // P      # 4  feature tiles of the residual stream
FT = FF // P       # 16 ff hidden tiles
CT = CIN // P      # 8  conv channel tiles
NCORES = 8
PAD = KW - 1       # 30 causal pad
S1POW = 10         # fp8 pre-scale exponent: w1/w3/c1/c2
S2POW = 11         # fp8 pre-scale exponent: w2/w4 (include the 0.5)
SDPOW = 9          # fp8 pre-scale exponent: depthwise taps (folded into bns)


def build(split_waits=True):
    """Build the single-core Bass module (SPMD: same NEFF on all 8 cores)."""
    import concourse.bass as bass
    import concourse.mybir as mybir
    import concourse.tile as tile

    F32 = mybir.dt.float32
    F32R = mybir.dt.float32r
    BF16 = mybir.dt.bfloat16
    AF = mybir.ActivationFunctionType
    AL = mybir.AluOpType

    nc = bass.Bass()

    # ---------------- I/O ----------------
    FP8 = mybir.dt.float8e4
    FP16 = mybir.dt.float16
    KP = KW + 1  # dw taps padded to 32 for even DoubleRow pairing

    xT_d = nc.dram_tensor("xT", [DIM, N], F32R, kind="ExternalInput")
    # fp8 DoubleRow-packed weights: [n_pairs, P, 2, cols], scaled by 2^S*
    w1_d = nc.dram_tensor("w1p", [2, P, 2, FF], FP8, kind="ExternalInput")
    w2_d = nc.dram_tensor("w2p", [FT // 2, P, 2, DIM], FP8, kind="ExternalInput")
    wq_d = nc.dram_tensor("wq", [DIM, INNER], F32R, kind="ExternalInput")
    wk_d = nc.dram_tensor("wk", [DIM, INNER], F32R, kind="ExternalInput")
    wv_d = nc.dram_tensor("wv", [DIM, INNER], F32R, kind="ExternalInput")
    bv_d = nc.dram_tensor("bvb", [P, INNER], F32R, kind="ExternalInput")
    wo_d = nc.dram_tensor("wo", [INNER, DIM], F32R, kind="ExternalInput")
    relT_d = nc.dram_tensor("relT", [P, 2 * MAXP + 1], F32R, kind="ExternalInput")
    c1_d = nc.dram_tensor("c1p", [2, P, 2, 2 * CIN], FP8, kind="ExternalInput")
    dwd_d = nc.dram_tensor("dwp", [CT, P, KP, P], FP8, kind="ExternalInput")
    c2_d = nc.dram_tensor("c2p", [CT // 2, P, 2, DIM], FP8, kind="ExternalInput")
    w3_d = nc.dram_tensor("w3p", [2, P, 2, FF], FP8, kind="ExternalInput")
    w4_d = nc.dram_tensor("w4p", [FT // 2, P, 2, DIM], FP8, kind="ExternalInput")
    antid_d = nc.dram_tensor("antid", [P, P], F32R, kind="ExternalInput")
    onesf_d = nc.dram_tensor("onesf", [P, P], F32R, kind="ExternalInput")
    onesm_d = nc.dram_tensor("onesm", [P, P], F32R, kind="ExternalInput")
    # packed small per-tile biases/affines: see prep_inputs for column map
    NPK = 96
    pk_d = nc.dram_tensor("cstpack", [P, NPK], F32, kind="ExternalInput")

    outT_d = nc.dram_tensor("outT", [DIM, N], F32, kind="ExternalOutput")

    QRW = 2 * MAXP + 1  # 1025 scratch row width
    qr_d = nc.dram_tensor("qr_scratch", [H, N, QRW], FP16, kind="Internal")

    def r32(ap):
        return ap.bitcast(F32R)

    DR = mybir.MatmulPerfMode.DoubleRow
    S1INV = 2.0 ** -S1POW   # w1/w3/c1/c2 pre-scale compensation
    S2INV = 2.0 ** -S2POW   # w2/w4 pre-scale compensation

    with tile.TileContext(nc) as tc:
        with (
            nc.allow_low_precision(reason="fp32r/bf16 matmul feeds"),
            tc.tile_pool(name="cst", bufs=1) as cst,
            tc.tile_pool(name="sb", bufs=2) as sb,
            tc.tile_pool(name="ps", bufs=2, space="PSUM") as psp,
        ):

            # ---------------- x + constants (x first: LN stats need only x) ---
            ones_full = cst.tile([P, P], F32R, tag="ones_full")
            nc.sync.dma_start(ones_full[:, :], onesf_d[:, :])
            ones_mean = cst.tile([P, P], F32R, tag="ones_mean")
            nc.sync.dma_start(ones_mean[:, :], onesm_d[:, :])
            xs = []
            for mt in range(DT):
                xt = sb.tile([P, N], F32R, tag="x", bufs=6)
                nc.sync.dma_start(xt[:, :], xT_d[mt * P:(mt + 1) * P, :])
                xs.append(xt)
            pkt = cst.tile([P, NPK], F32, tag="cstpack")
            nc.sync.dma_start(pkt[:, :], pk_d[:, :])
            b1t = pkt[:, 0:16]
            b2t = pkt[:, 16:20]
            bqt = pkt[:, 20:24]
            bkt = pkt[:, 24:28]
            bot = pkt[:, 28:32]
            c1at = pkt[:, 32:40]
            c1gt = pkt[:, 40:48]
            bnst = pkt[:, 48:56]
            bntt = pkt[:, 56:64]
            c2bt = pkt[:, 64:68]
            b3t = pkt[:, 68:84]
            b4t = pkt[:, 84:88]
            pngt = pkt[:, 88:92]
            pnbt = pkt[:, 92:96]
            relT = cst.tile([P, QRW], F32R, tag="relT")
            nc.sync.dma_start(relT[:, :], relT_d[:, :])
            bvt = cst.tile([P, INNER], F32R, tag="bvt")
            nc.sync.dma_start(bvt[:, :], bv_d[:, :])

            # ---------------- helpers ----------------
            def layer_norm_rc(xin):
                """Stats of LN over the partition (feature) axis.

                Returns r_b, c_b [128, 512] tiles with z = x*r_b + c_b.
                The 1/DIM is folded into the ones_mean stationary; squares run
                on the otherwise-idle GpSimd engine."""
                ps_mean = psp.tile([P, N], F32, tag="mm", bufs=2)
                for kt in range(DT):
                    nc.tensor.matmul(ps_mean[:, :], ones_mean[:, :], xin[kt][:, :],
                                     start=(kt == 0), stop=(kt == DT - 1))
                ps_sq = psp.tile([P, N], F32, tag="mm", bufs=2)
                for kt in range(DT):
                    xsq = sb.tile([P, N], F32R, tag="tmp", bufs=3)
                    nc.scalar.square(xsq[:, :], xin[kt][:, :])
                    nc.tensor.matmul(ps_sq[:, :], ones_mean[:, :], xsq[:, :],
                                     start=(kt == 0), stop=(kt == DT - 1))
                nm2 = sb.tile([P, N], F32, tag="tmp", bufs=3)
                nc.scalar.activation(nm2[:, :], ps_mean[:, :], AF.Square)
                veps = sb.tile([P, N], F32, tag="tmp", bufs=3)
                nc.vector.scalar_tensor_tensor(veps[:, :], ps_sq[:, :], EPS,
                                               nm2[:, :], AL.add, AL.subtract)
                lnv = sb.tile([P, N], F32, tag="tmp", bufs=3)
                nc.scalar.activation(lnv[:, :], veps[:, :], AF.Ln)
                r_b = sb.tile([P, N], F32, tag="r_b", bufs=2)
                nc.scalar.activation(r_b[:, :], lnv[:, :], AF.Exp, scale=-0.5)
                c_b = sb.tile([P, N], F32, tag="c_b", bufs=2)
                nc.vector.scalar_tensor_tensor(c_b[:, :], ps_mean[:, :], -1.0,
                                               r_b[:, :], AL.mult, AL.mult)
                return r_b, c_b

            def ln_apply(xin, r_b, c_b):
                zs = []
                for kt in range(DT):
                    t = sb.tile([P, N], F32, tag="lnt", bufs=2)
                    nc.vector.tensor_mul(t[:, :], xin[kt][:, :], r_b[:, :])
                    z = sb.tile([P, N], F32R, tag="z", bufs=4)
                    nc.vector.tensor_add(z[:, :], t[:, :], c_b[:, :])
                    zs.append(z)
                return zs

            def ln_apply8(xin, r_b, c_b):
                """LN apply straight into fp8 pair tiles (mm1 moving); two
                tiles so the first DoubleRow pair can start after 2 writes."""
                z8s = [sb.tile([P, 2, N], FP8, tag="z8", bufs=4,
                               name=f"z8_{i}") for i in range(2)]
                for kt in range(DT):
                    t = sb.tile([P, N], F32, tag="lnt", bufs=2)
                    nc.vector.tensor_mul(t[:, :], xin[kt][:, :], r_b[:, :])
                    nc.vector.tensor_add(z8s[kt // 2][:, kt % 2, :], t[:, :],
                                         c_b[:, :])
                return z8s

            def ff_block(xin, w_d, bt, w2p_d, b2tt):
                """x + 0.5*ff(LN(x)); fp8 DoubleRow matmuls; returns new
                residual tiles.  Weights pre-scaled 2^S1 / 2^S2 on host."""
                r_b, c_b = layer_norm_rc(xin)
                z8s = ln_apply8(xin, r_b, c_b)
                # h = silu(z @ w1 * 2^-S1 + b1) directly on ACT, fp8 out
                w1ts = []
                for q in range(2):
                    wt = sb.tile([P, 2, FF], FP8, tag="w8", bufs=3)
                    nc.sync.dma_start(wt[:, :, :], w_d[q])
                    w1ts.append(wt)
                h8s = [sb.tile([P, FT // 2, N], FP8, tag="h8", bufs=2,
                               name=f"h8_{i}") for i in range(2)]
                for mt in range(FT):
                    ph = psp.tile([P, N], F32, tag="acc", bufs=4)
                    for q in range(2):
                        nc.tensor.matmul(ph[:, :],
                                         w1ts[q][:, :, mt * P:(mt + 1) * P],
                                         z8s[q][:, :, :],
                                         start=(q == 0), stop=(q == 1),
                                         perf_mode=DR)
                    nc.scalar.activation(h8s[mt // 8][:, mt % 8, :], ph[:, :],
                                         AF.Silu, bias=bt[:, mt:mt + 1],
                                         scale=S1INV)
                # y = h @ w2 * 2^-S2, pair-outer with 4 psum accumulators
                pys = [psp.tile([P, N], F32, tag="acc", bufs=4, name=f"pys{i}") for i in range(DT)]
                for r in range(FT // 2):
                    wt = sb.tile([P, 2, DIM], FP8, tag="w8s", bufs=10)
                    nc.sync.dma_start(wt[:, :, :], w2p_d[r])
                    rhs = h8s[(2 * r) // 8][:, (2 * r) % 8:(2 * r) % 8 + 2, :]
                    for mt in range(DT):
                        nc.tensor.matmul(pys[mt][:, :],
                                         wt[:, :, mt * P:(mt + 1) * P], rhs,
                                         start=(r == 0), stop=(r == FT // 2 - 1),
                                         perf_mode=DR)
                xo = []
                for mt in range(DT):
                    tb = sb.tile([P, N], F32, tag="lnt", bufs=2)
                    nc.scalar.activation(tb[:, :], pys[mt][:, :], AF.Identity,
                                         bias=b2tt[:, mt:mt + 1], scale=S2INV)
                    t = sb.tile([P, N], F32R, tag="x", bufs=6)
                    nc.vector.tensor_add(t[:, :], tb[:, :], xin[mt][:, :])
                    xo.append(t)
                return xo

            # ================= ff1 =================
            x1 = ff_block(xs, w1_d, b1t, w2_d, b2t)

            # ================= attention =================
            r_b, c_b = layer_norm_rc(x1)
            zs = ln_apply(x1, r_b, c_b)

            def proj_qk(w_dram, bias_t, tag):
                wts = []
                for kt in range(DT):
                    wt = sb.tile([P, INNER], F32R, tag="wsm", bufs=4)
                    nc.sync.dma_start(wt[:, :], w_dram[kt * P:(kt + 1) * P, :])
                    wts.append(wt)
                outs = []
                for mt in range(DT):
                    pq = psp.tile([P, N], F32, tag="mm", bufs=2)
                    for kt in range(DT):
                        nc.tensor.matmul(pq[:, :], r32(wts[kt][:, mt * P:(mt + 1) * P]),
                                         r32(zs[kt][:, :]),
                                         start=(kt == 0), stop=(kt == DT - 1))
                    qt = sb.tile([P, N], F32R, tag=tag, bufs=4)
                    nc.vector.tensor_scalar(out=qt[:, :], in0=pq[:, :],
                                            scalar1=bias_t[:, mt:mt + 1], scalar2=None,
                                            op0=AL.add)
                    outs.append(qt)
                return outs

            qTs = proj_qk(wq_d, bqt, "qT")
            kTs = proj_qk(wk_d, bkt, "kT")

            # v in time-major layout with a trailing ones column per head
            wvts = []
            for kt in range(DT):
                wt = sb.tile([P, INNER], F32R, tag="wsm", bufs=4)
                nc.sync.dma_start(wt[:, :], wv_d[kt * P:(kt + 1) * P, :])
                wvts.append(wt)
            vext = []
            for jt in range(DT):
                pv = psp.tile([P, N], F32, tag="mm", bufs=2)
                for kt in range(DT):
                    nc.tensor.matmul(pv[:, :], r32(zs[kt][:, jt * P:(jt + 1) * P]),
                                     r32(wvts[kt][:, :]),
                                     start=(kt == 0), stop=(kt == DT - 1))
                vx = sb.tile([P, H * 2 * DH], F32R, tag="vext", bufs=4)
                vw = vx[:, 0:H * 2 * DH].rearrange("p (h c) -> p h c", c=2 * DH)
                nc.vector.scalar_tensor_tensor(
                    vw[:, :, 0:DH],
                    pv[:, :].rearrange("p (h d) -> p h d", h=H), 1.0,
                    bvt[:, :].rearrange("p (h d) -> p h d", h=H),
                    AL.mult, AL.add)
                nc.vector.tensor_copy(
                    vw[:, :, DH:2 * DH],
                    ones_full[:, None, 0:DH].broadcast_to([P, H, DH]))
                vext.append(vx)

            # qr bounce (bf16 scratch) + scores, software-pipelined two heads
            # deep: while head h computes scores, head h+2's qr windows are
            # written (Scalar/GpSimd queues) and head h+1's rel tiles stream
            # back (Sync queue).
            def emit_qr(h):
                hb = (h % 2) * DH
                for it in range(DT):
                    lq = qTs[h // 2][hb:hb + DH, it * P:(it + 1) * P]
                    cr0 = 3 * P - P * it
                    pq1 = psp.tile([P, 320], F32, tag="acc", bufs=4)
                    nc.tensor.matmul(pq1[:, :], r32(lq),
                                     r32(relT[hb:hb + DH, cr0:cr0 + 320]),
                                     start=True, stop=True)
                    pq2 = psp.tile([P, 320], F32, tag="acc", bufs=4)
                    nc.tensor.matmul(pq2[:, :], r32(lq),
                                     r32(relT[hb:hb + DH, cr0 + 320:cr0 + 640]),
                                     start=True, stop=True)
                    qt = sb.tile([P, 640], FP16, tag="qt", bufs=3)
                    nc.vector.tensor_copy(qt[:, 0:320], pq1[:, :])
                    nc.vector.tensor_copy(qt[:, 320:640], pq2[:, :])
                    nc.gpsimd.dma_start(
                        qr_d[h, it * P:(it + 1) * P, cr0:cr0 + 640], qt[:, :])

            def emit_rel_reads(h):
                """One XBAR-transposing DMA per jt: rel_T[j, i] tiles direct
                from the scratch (globally qr[i, i-j+512] = addr i*1024 + j).
                Issue cost ~1.2us each, so spread 3:1 over Sync and Scalar."""
                rels = []
                for jt in range(DT):
                    rel = sb.tile([P, N], FP16, tag="rel", bufs=12)
                    src = bass.AP(qr_d, h * N * QRW + 4 * P + jt * P,
                                  [[QRW - 1, N], [1, P]])
                    eng = nc.scalar if jt == 3 else nc.sync
                    eng.dma_start_transpose(rel[:, :], src)
                    rels.append(rel)
                return rels

            oTs = [sb.tile([P, N], F32R, tag="oT", bufs=4, name=f"oTs{i}") for i in range(DT)]
            emit_qr(0)
            emit_qr(1)
            emit_qr(2)
            rel_q = [emit_rel_reads(0), emit_rel_reads(1)]
            for h in range(H):
                hb = (h % 2) * DH
                rels = rel_q.pop(0)
                if h + 3 < H:
                    emit_qr(h + 3)
                if h + 2 < H:
                    rel_q.append(emit_rel_reads(h + 2))
                exps = []
                for jt in range(DT):
                    pss = psp.tile([P, N], F32, tag="acc", bufs=4)
                    nc.tensor.matmul(pss[:, :],
                                     r32(kTs[h // 2][hb:hb + DH, jt * P:(jt + 1) * P]),
                                     r32(qTs[h // 2][hb:hb + DH, :]),
                                     start=True, stop=True)
                    epre = sb.tile([P, N], F32, tag="tmp", bufs=3)
                    nc.vector.tensor_add(epre[:, :], pss[:, :], rels[jt][:, :])
                    e = sb.tile([P, N], F32R, tag="exp", bufs=5)
                    nc.scalar.activation(e[:, :], epre[:, :], AF.Exp)
                    exps.append(e)
                po = psp.tile([P, N], F32, tag="mm", bufs=2)
                for jt in range(DT):
                    nc.tensor.matmul(po[:, :],
                                     r32(vext[jt][:, h * 2 * DH:(h + 1) * 2 * DH]),
                                     r32(exps[jt][:, :]),
                                     start=(jt == 0), stop=(jt == DT - 1))
                lnd = sb.tile([DH, N], F32, tag="dwt", bufs=2)
                nc.scalar.activation(lnd[:, :], po[DH:2 * DH, :], AF.Ln)
                rb = sb.tile([DH, N], F32, tag="dwt", bufs=2)
                nc.scalar.activation(rb[:, :], lnd[:, :], AF.Exp, scale=-1.0)
                nc.vector.tensor_mul(oTs[h // 2][hb:hb + DH, :], po[0:DH, :],
                                     rb[:, :])

            # out-projection + residual
            wots = []
            for kt in range(DT):
                wt = sb.tile([P, DIM], F32R, tag="wsm", bufs=4)
                nc.sync.dma_start(wt[:, :], wo_d[kt * P:(kt + 1) * P, :])
                wots.append(wt)
            pas = [psp.tile([P, N], F32, tag="acc", bufs=4, name=f"pas{i}") for i in range(DT)]
            for kt in range(DT):
                for mt in range(DT):
                    nc.tensor.matmul(pas[mt][:, :], r32(wots[kt][:, mt * P:(mt + 1) * P]),
                                     r32(oTs[kt][:, :]),
                                     start=(kt == 0), stop=(kt == DT - 1))
            x2 = []
            for mt in range(DT):
                t = sb.tile([P, N], F32R, tag="x", bufs=6)
                nc.vector.scalar_tensor_tensor(t[:, :], pas[mt][:, :],
                                               bot[:, mt:mt + 1], x1[mt][:, :],
                                               AL.add, AL.add)
                x2.append(t)

            # ================= conv module =================
            # conv1 (fp8 DoubleRow): cast x2 to fp8, natural [a | g] col order
            x28s = [sb.tile([P, 2, N], FP8, tag="z8", bufs=4,
                            name=f"x28_{i}") for i in range(2)]
            for kt in range(DT):
                nc.vector.tensor_copy(x28s[kt // 2][:, kt % 2, :], x2[kt][:, :])
            c1ts = []
            for q in range(2):
                wt = sb.tile([P, 2, 2 * CIN], FP8, tag="w8", bufs=3)
                nc.sync.dma_start(wt[:, :, :], c1_d[q])
                c1ts.append(wt)
            GW = PAD + N + 2  # 544: causal pad + time + pad-to-32-taps slack
            glus = []
            for ct in range(CT):
                pa = psp.tile([P, N], F32, tag="acc", bufs=4)
                pg = psp.tile([P, N], F32, tag="acc", bufs=4)
                for q in range(2):
                    nc.tensor.matmul(pa[:, :],
                                     c1ts[q][:, :, ct * P:(ct + 1) * P],
                                     x28s[q][:, :, :],
                                     start=(q == 0), stop=(q == 1), perf_mode=DR)
                for q in range(2):
                    nc.tensor.matmul(pg[:, :],
                                     c1ts[q][:, :, CIN + ct * P:CIN + (ct + 1) * P],
                                     x28s[q][:, :, :],
                                     start=(q == 0), stop=(q == 1), perf_mode=DR)
                sig = sb.tile([P, N], F32, tag="tmp", bufs=3)
                nc.scalar.activation(sig[:, :], pg[:, :], AF.Sigmoid,
                                     bias=c1gt[:, ct:ct + 1], scale=S1INV)
                ya = sb.tile([P, N], F32, tag="tmp", bufs=3)
                nc.scalar.activation(ya[:, :], pa[:, :], AF.Identity,
                                     bias=c1at[:, ct:ct + 1], scale=S1INV)
                glu = sb.tile([P, GW], FP8, tag="glu8", bufs=8)
                nc.vector.memset(glu[:, 0:PAD], 0.0)
                nc.vector.memset(glu[:, PAD + N:GW], 0.0)
                nc.vector.tensor_mul(glu[:, PAD:PAD + N], ya[:, :], sig[:, :])
                glus.append(glu)

            # depthwise conv: 16 DoubleRow tap-pair matmuls per channel block
            # (tap 31 is a zero diagonal).  Moving operand is an overlapping
            # [P, 2, N] view of the padded glu tile (stride-1 tap shift).
            hc8s = [sb.tile([P, CT // 2, N], FP8, tag="h8", bufs=2,
                            name=f"hc8_{i}") for i in range(2)]
            for ct in range(CT):
                dg = sb.tile([P, KP, P], FP8, tag="dg8", bufs=3)
                nc.sync.dma_start(dg[:, :, :], dwd_d[ct])
                pd = psp.tile([P, N], F32, tag="mm", bufs=2)
                gbase = glus[ct][:, 0:GW]
                for r in range(KP // 2):
                    rhs = bass.AP(gbase.tensor, gbase.offset + 2 * r,
                                  [[GW, P], [1, 2], [1, N]])
                    nc.tensor.matmul(pd[:, :], dg[:, 2 * r:2 * r + 2, :], rhs,
                                     start=(r == 0), stop=(r == KP // 2 - 1),
                                     perf_mode=DR)
                sig = sb.tile([P, N], F32, tag="dwt", bufs=2)
                nc.scalar.activation(sig[:, :], pd[:, :], AF.Sigmoid,
                                     bias=bntt[:, ct:ct + 1], scale=bnst[:, ct:ct + 1])
                u = sb.tile([P, N], F32, tag="dwt", bufs=2)
                nc.vector.tensor_scalar(out=u[:, :], in0=pd[:, :],
                                        scalar1=bnst[:, ct:ct + 1],
                                        scalar2=bntt[:, ct:ct + 1],
                                        op0=AL.mult, op1=AL.add)
                nc.vector.tensor_mul(hc8s[ct // 4][:, ct % 4, :], u[:, :],
                                     sig[:, :])

            # conv2 (fp8 DoubleRow) + residual
            c2ts = []
            for q in range(CT // 2):
                wt = sb.tile([P, 2, DIM], FP8, tag="w8s", bufs=10)
                nc.sync.dma_start(wt[:, :, :], c2_d[q])
                c2ts.append(wt)
            pcs = [psp.tile([P, N], F32, tag="acc", bufs=4, name=f"pcs{i}") for i in range(DT)]
            for q in range(CT // 2):
                rhs = hc8s[q // 2][:, (2 * q) % 4:(2 * q) % 4 + 2, :]
                for mt in range(DT):
                    nc.tensor.matmul(pcs[mt][:, :],
                                     c2ts[q][:, :, mt * P:(mt + 1) * P],
                                     rhs,
                                     start=(q == 0), stop=(q == CT // 2 - 1),
                                     perf_mode=DR)
            x3 = []
            for mt in range(DT):
                tb = sb.tile([P, N], F32, tag="lnt", bufs=2)
                nc.scalar.activation(tb[:, :], pcs[mt][:, :], AF.Identity,
                                     bias=c2bt[:, mt:mt + 1], scale=S1INV)
                t = sb.tile([P, N], F32R, tag="x", bufs=6)
                nc.vector.tensor_add(t[:, :], tb[:, :], x2[mt][:, :])
                x3.append(t)

            # ================= ff2 =================
            x4 = ff_block(x3, w3_d, b3t, w4_d, b4t)

            # ================= post-LN =================
            r_b, c_b = layer_norm_rc(x4)
            for mt in range(DT):
                t = sb.tile([P, N], F32, tag="lnt", bufs=2)
                nc.vector.tensor_mul(t[:, :], x4[mt][:, :], r_b[:, :])
                t2 = sb.tile([P, N], F32, tag="lnt2", bufs=1)
                nc.vector.tensor_add(t2[:, :], t[:, :], c_b[:, :])
                ot = sb.tile([P, N], F32, tag="outt", bufs=2)
                nc.vector.tensor_scalar(out=ot[:, :], in0=t2[:, :],
                                        scalar1=pngt[:, mt:mt + 1],
                                        scalar2=pnbt[:, mt:mt + 1],
                                        op0=AL.mult, op1=AL.add)
                eng = (nc.scalar, nc.sync, nc.gpsimd, nc.scalar)[mt]
                eng.dma_start(outT_d[mt * P:(mt + 1) * P, :], ot[:, :])

    if split_waits:
        _split_matmul_waits(nc, mybir)
    return nc


def _split_matmul_waits(nc, mybir):
    """This walrus build rejects engine instructions carrying more than one
    sync wait; hoist the extras onto EventSemaphore instructions on the same
    engine queue right before the instruction."""
    fn = nc.m.functions[0]
    ctr = 0
    for blk in fn.blocks:
        out = []
        changed = False
        for ins in blk.instructions:
            si = ins.sync_info
            if (si is not None and si.on_wait and len(si.on_wait) > 1
                    and not isinstance(ins, (mybir.InstEventSemaphore,
                                             mybir.InstNoOp))):
                waits = list(si.on_wait)
                for w in waits[:-1]:
                    ev = mybir.InstNoOp(
                        name=f"EVW-{ctr}", ins=[], outs=[],
                        sync_info=mybir.SyncInfo(on_wait=[w], on_update=[]))
                    ev.engine = ins.engine
                    ctr += 1
                    out.append(ev)
                ins.sync_info = mybir.SyncInfo(
                    on_wait=[waits[-1]], on_update=list(si.on_update or []))
                changed = True
            out.append(ins)
        if changed:
            blk.instructions = out


def prep_inputs(inputs):
    """Host-side preprocessing: fold LN affines / scales / biases into weights."""
    import ml_dtypes

    f = np.float32
    f8 = ml_dtypes.float8_e4m3
    ii = {k: np.asarray(v, dtype=f) for k, v in inputs.items()}

    def colmaj(b, nb):
        return np.ascontiguousarray(b.astype(f).reshape(nb, P).T)

    def pack_pairs(w, spow):
        """[K, M] weight -> [K//256, P, 2, M] fp8, scaled by 2^spow."""
        K, M = w.shape
        t = (w * (2.0 ** spow)).reshape(K // (2 * P), 2, P, M).transpose(0, 2, 1, 3)
        return np.ascontiguousarray(t.astype(f8))

    g1, be1 = ii["ff1_ln_g"], ii["ff1_ln_b"]
    w1 = pack_pairs(g1[:, None] * ii["ff1_w1"], S1POW)
    b1 = colmaj(be1 @ ii["ff1_w1"] + ii["ff1_b1"], FT)
    w2 = pack_pairs(0.5 * ii["ff1_w2"], S2POW)
    b2 = colmaj(0.5 * ii["ff1_b2"], DT)

    ag, ab = ii["attn_ln_g"], ii["attn_ln_b"]
    sc = DH ** -0.5
    wq = np.ascontiguousarray(ag[:, None] * ii["wq"] * sc)
    bq = colmaj((ab @ ii["wq"] + ii["bq"]) * sc, DT)
    wkv, bkv = ii["wkv"], ii["bkv"]
    wk = np.ascontiguousarray(ag[:, None] * wkv[:, :INNER])
    bk = colmaj(ab @ wkv[:, :INNER] + bkv[:INNER], DT)
    wv = np.ascontiguousarray(ag[:, None] * wkv[:, INNER:])
    bv = np.ascontiguousarray(np.broadcast_to(
        ab @ wkv[:, INNER:] + bkv[INNER:], (P, INNER)))
    wo = np.ascontiguousarray(ii["wo"])
    bo = colmaj(ii["bo"], DT)
    # relT rows: head feature d lives at partition (h%2)*64 + d -> duplicate rows
    rT = ii["rel_emb"].T[:, ::-1]  # [64, 1025] column-reversed
    relT = np.ascontiguousarray(np.concatenate([rT, rT], axis=0))  # [128, 1025]

    # conv1 in natural [a(0:CIN) | g(CIN:2CIN)] column order
    c1 = pack_pairs(ii["conv1_w"], S1POW)
    c1b = ii["conv1_b"]
    c1a = colmaj(c1b[:CIN], CT)
    c1g = colmaj(c1b[CIN:], CT)
    # depthwise taps as diagonal pair blocks [CT, P, 32, P] (tap 31 zero)
    KP = KW + 1
    dwd = np.zeros((CT, P, KP, P), dtype=f)
    wr = ii["dw_w"].reshape(CT, P, KW) * (2.0 ** SDPOW)
    pp = np.arange(P)
    for ct in range(CT):
        for k in range(KW):
            dwd[ct, pp, k, pp] = wr[ct, :, k]
    dwd = np.ascontiguousarray(dwd.astype(f8))
    inv = 1.0 / np.sqrt(ii["bn_var"] + EPS)
    s = inv * ii["bn_g"]
    t = ii["bn_b"] - ii["bn_mean"] * s
    bns = colmaj(s * (2.0 ** -SDPOW), CT)
    bnt = colmaj(t + s * ii["dw_b"], CT)
    c2 = pack_pairs(ii["conv2_w"], S1POW)
    c2b = colmaj(ii["conv2_b"], DT)

    g3, be3 = ii["ff2_ln_g"], ii["ff2_ln_b"]
    w3 = pack_pairs(g3[:, None] * ii["ff2_w1"], S1POW)
    b3 = colmaj(be3 @ ii["ff2_w1"] + ii["ff2_b1"], FT)
    w4 = pack_pairs(0.5 * ii["ff2_w2"], S2POW)
    b4 = colmaj(0.5 * ii["ff2_b2"], DT)

    png = colmaj(ii["pn_g"], DT)
    pnb = colmaj(ii["pn_b"], DT)

    # pack the small [P, k] constants into one tensor; column map mirrors build()
    pk = np.concatenate([b1, b2, bq, bk, bo, c1a, c1g, bns, bnt, c2b,
                         b3, b4, png, pnb], axis=1)
    assert pk.shape == (P, 96), pk.shape
    shared = dict(w1p=w1, w2p=w2, wq=wq, wk=wk,
                  wv=wv, bvb=bv, wo=wo, relT=relT, c1p=c1,
                  dwp=dwd, c2p=c2,
                  w3p=w3, w4p=w4,
                  cstpack=np.ascontiguousarray(pk),
                  antid=np.ascontiguousarray(np.eye(P, dtype=f)),
                  onesf=np.ones((P, P), dtype=f),
                  onesm=np.full((P, P), 1.0 / DIM, dtype=f))
    x = ii["x"]
    in_maps = []
    for b in range(NCORES):
        m = dict(shared)
        m["xT"] = np.ascontiguousarray(x[b].T)
        in_maps.append(m)
    return in_maps


_BUILT = None


def run(inputs, trace=False):
    global _BUILT
    from concourse import bass_utils

    in_maps = prep_inputs(inputs)
    if _BUILT is None:
        _BUILT = build()
    res = bass_utils.run_bass_kernel_spmd(
        _BUILT, in_maps, core_ids=list(range(NCORES)), trace=trace)
    out = np.stack([np.asarray(r["outT"]).T for r in res.results])
    return np.ascontiguousarray(out.astype(np.float32)), res


def kernel(**inputs):
    out, _ = run(inputs, trace=False)
    return out



# revision 54
# speedup vs baseline: 1.0038x; 1.0038x over previous
"""Trainium2 Bass kernel: Conformer block (B=8, N=512, DIM=512, H=8, DH=64, FF=2048, CIN=1024, K=31).

Sharding: pure data-parallel over batch — each of the 8 NeuronCores processes one
batch item with the full weight set (no collectives).

Layout: activations are kept FEATURE-major ([feature, time] = x.T) on chip so that
chained matmuls need no transposes (weights stay in natural [din, dout] layout as
the stationary operand).  LayerNorm reductions over features become ones-vector
matmuls on the PE; per-time-step affine factors are broadcast across partitions
with a GPSIMD partition_broadcast.

Relative-position attention uses the shift-gather trick: qr = q @ rel_emb.T is
bounced through an internal DRAM scratch and read back with a strided
(stride = row+1, step -1) access pattern so that rel[j, i] = qr[i, i-j+512]
lands directly as the transposed score tile.  Scores are computed transposed
(dots_T[j, i]) so softmax runs over the partition axis: exp on ACT, the
denominator via a ones-column fused into the attn@v matmul, and the final
normalization as a broadcasted multiply.

The causal depthwise conv runs on the PE as 31 PSUM-accumulated matmuls per
128-channel block against diagonal stationary matrices; the diagonals are
(re)written with a single strided DMA per block (dst step = row+1).

Matmuls use float32r (1 cycle/row for N>=256); the FFN second matmul and the
depthwise conv run in bf16.
"""

import sys

for _p in ("/opt/trn_rl_repo", "/root/.axon_site/_ro/trn_rl_repo"):
    if _p not in sys.path:
        sys.path.insert(0, _p)

import numpy as np

B, N, DIM, H, DH, MULT, EXP, KW, MAXP = 8, 512, 512, 8, 64, 4, 2, 31, 512
INNER = H * DH
FF = DIM * MULT
CIN = DIM * EXP
EPS = 1e-5
P = 128
DT = DIM // P      # 4  feature tiles of the residual stream
FT = FF // P       # 16 ff hidden tiles
CT = CIN // P      # 8  conv channel tiles
NCORES = 8
PAD = KW - 1       # 30 causal pad
S1POW = 10         # fp8 pre-scale exponent: w1/w3/c1/c2
S2POW = 11         # fp8 pre-scale exponent: w2/w4 (include the 0.5)
SDPOW = 9          # fp8 pre-scale exponent: depthwise taps (folded into bns)


def build(split_waits=True):
    """Build the single-core Bass module (SPMD: same NEFF on all 8 cores)."""
    import concourse.bass as bass
    import concourse.mybir as mybir
    import concourse.tile as tile

    F32 = mybir.dt.float32
    F32R = mybir.dt.float32r
    BF16 = mybir.dt.bfloat16
    AF = mybir.ActivationFunctionType
    AL = mybir.AluOpType

    nc = bass.Bass()

    # ---------------- I/O ----------------
    FP8 = mybir.dt.float8e4
    FP16 = mybir.dt.float16
    KP = KW + 1  # dw taps padded to 32 for even DoubleRow pairing

    xT_d = nc.dram_tensor("xT", [DIM, N], F32R, kind="ExternalInput")
    # fp8 DoubleRow-packed weights: [n_pairs, P, 2, cols], scaled by 2^S*
    w1_d = nc.dram_tensor("w1p", [2, P, 2, FF], FP8, kind="ExternalInput")
    w2_d = nc.dram_tensor("w2p", [FT // 2, P, 2, DIM], FP8, kind="ExternalInput")
    wq_d = nc.dram_tensor("wq", [DIM, INNER], F32R, kind="ExternalInput")
    wk_d = nc.dram_tensor("wk", [DIM, INNER], F32R, kind="ExternalInput")
    wv_d = nc.dram_tensor("wv", [DIM, INNER], F32R, kind="ExternalInput")
    bv_d = nc.dram_tensor("bvb", [P, INNER], F32R, kind="ExternalInput")
    wo_d = nc.dram_tensor("wo", [INNER, DIM], F32R, kind="ExternalInput")
    relT_d = nc.dram_tensor("relT", [P, 2 * MAXP + 1], F32R, kind="ExternalInput")
    c1_d = nc.dram_tensor("c1p", [2, P, 2, 2 * CIN], FP8, kind="ExternalInput")
    dwd_d = nc.dram_tensor("dwp", [CT, P, KP, P], FP8, kind="ExternalInput")
    c2_d = nc.dram_tensor("c2p", [CT // 2, P, 2, DIM], FP8, kind="ExternalInput")
    w3_d = nc.dram_tensor("w3p", [2, P, 2, FF], FP8, kind="ExternalInput")
    w4_d = nc.dram_tensor("w4p", [FT // 2, P, 2, DIM], FP8, kind="ExternalInput")
    antid_d = nc.dram_tensor("antid", [P, P], F32R, kind="ExternalInput")
    onesf_d = nc.dram_tensor("onesf", [P, P], F32R, kind="ExternalInput")
    onesm_d = nc.dram_tensor("onesm", [P, P], F32R, kind="ExternalInput")
    # packed small per-tile biases/affines: see prep_inputs for column map
    NPK = 96
    pk_d = nc.dram_tensor("cstpack", [P, NPK], F32, kind="ExternalInput")

    outT_d = nc.dram_tensor("outT", [DIM, N], F32, kind="ExternalOutput")

    QRW = 2 * MAXP + 1  # 1025 scratch row width
    qr_d = nc.dram_tensor("qr_scratch", [H, N, QRW], FP16, kind="Internal")

    def r32(ap):
        return ap.bitcast(F32R)

    DR = mybir.MatmulPerfMode.DoubleRow
    S1INV = 2.0 ** -S1POW   # w1/w3/c1/c2 pre-scale compensation
    S2INV = 2.0 ** -S2POW   # w2/w4 pre-scale compensation

    with tile.TileContext(nc) as tc:
        with (
            nc.allow_low_precision(reason="fp32r/bf16 matmul feeds"),
            tc.tile_pool(name="cst", bufs=1) as cst,
            tc.tile_pool(name="sb", bufs=2) as sb,
            tc.tile_pool(name="ps", bufs=2, space="PSUM") as psp,
        ):

            # ---------------- x + constants (x first: LN stats need only x) ---
            ones_full = cst.tile([P, P], F32R, tag="ones_full")
            nc.sync.dma_start(ones_full[:, :], onesf_d[:, :])
            ones_mean = cst.tile([P, P], F32R, tag="ones_mean")
            nc.sync.dma_start(ones_mean[:, :], onesm_d[:, :])
            xs = []
            for mt in range(DT):
                xt = sb.tile([P, N], F32R, tag="x", bufs=6)
                nc.sync.dma_start(xt[:, :], xT_d[mt * P:(mt + 1) * P, :])
                xs.append(xt)
            pkt = cst.tile([P, NPK], F32, tag="cstpack")
            nc.sync.dma_start(pkt[:, :], pk_d[:, :])
            b1t = pkt[:, 0:16]
            b2t = pkt[:, 16:20]
            bqt = pkt[:, 20:24]
            bkt = pkt[:, 24:28]
            bot = pkt[:, 28:32]
            c1at = pkt[:, 32:40]
            c1gt = pkt[:, 40:48]
            bnst = pkt[:, 48:56]
            bntt = pkt[:, 56:64]
            c2bt = pkt[:, 64:68]
            b3t = pkt[:, 68:84]
            b4t = pkt[:, 84:88]
            pngt = pkt[:, 88:92]
            pnbt = pkt[:, 92:96]
            relT = cst.tile([P, QRW], F32R, tag="relT")
            nc.sync.dma_start(relT[:, :], relT_d[:, :])
            bvt = cst.tile([P, INNER], F32R, tag="bvt")
            nc.sync.dma_start(bvt[:, :], bv_d[:, :])

            # ---------------- helpers ----------------
            def layer_norm_rc(xin):
                """Stats of LN over the partition (feature) axis.

                Returns r_b, c_b [128, 512] tiles with z = x*r_b + c_b.
                The 1/DIM is folded into the ones_mean stationary; squares run
                on the otherwise-idle GpSimd engine."""
                ps_mean = psp.tile([P, N], F32, tag="mm", bufs=2)
                for kt in range(DT):
                    nc.tensor.matmul(ps_mean[:, :], ones_mean[:, :], xin[kt][:, :],
                                     start=(kt == 0), stop=(kt == DT - 1))
                ps_sq = psp.tile([P, N], F32, tag="mm", bufs=2)
                for kt in range(DT):
                    xsq = sb.tile([P, N], F32R, tag="tmp", bufs=3)
                    nc.scalar.square(xsq[:, :], xin[kt][:, :])
                    nc.tensor.matmul(ps_sq[:, :], ones_mean[:, :], xsq[:, :],
                                     start=(kt == 0), stop=(kt == DT - 1))
                nm2 = sb.tile([P, N], F32, tag="tmp", bufs=3)
                nc.scalar.activation(nm2[:, :], ps_mean[:, :], AF.Square)
                veps = sb.tile([P, N], F32, tag="tmp", bufs=3)
                nc.vector.scalar_tensor_tensor(veps[:, :], ps_sq[:, :], EPS,
                                               nm2[:, :], AL.add, AL.subtract)
                lnv = sb.tile([P, N], F32, tag="tmp", bufs=3)
                nc.scalar.activation(lnv[:, :], veps[:, :], AF.Ln)
                r_b = sb.tile([P, N], F32, tag="r_b", bufs=2)
                nc.scalar.activation(r_b[:, :], lnv[:, :], AF.Exp, scale=-0.5)
                c_b = sb.tile([P, N], F32, tag="c_b", bufs=2)
                nc.vector.scalar_tensor_tensor(c_b[:, :], ps_mean[:, :], -1.0,
                                               r_b[:, :], AL.mult, AL.mult)
                return r_b, c_b

            def ln_apply(xin, r_b, c_b):
                zs = []
                for kt in range(DT):
                    t = sb.tile([P, N], F32, tag="lnt", bufs=2)
                    nc.vector.tensor_mul(t[:, :], xin[kt][:, :], r_b[:, :])
                    z = sb.tile([P, N], F32R, tag="z", bufs=4)
                    nc.vector.tensor_add(z[:, :], t[:, :], c_b[:, :])
                    zs.append(z)
                return zs

            def ln_apply8(xin, r_b, c_b):
                """LN apply straight into fp8 pair tiles (mm1 moving); two
                tiles so the first DoubleRow pair can start after 2 writes."""
                z8s = [sb.tile([P, 2, N], FP8, tag="z8", bufs=4,
                               name=f"z8_{i}") for i in range(2)]
                for kt in range(DT):
                    t = sb.tile([P, N], F32, tag="lnt", bufs=2)
                    nc.vector.tensor_mul(t[:, :], xin[kt][:, :], r_b[:, :])
                    nc.vector.tensor_add(z8s[kt // 2][:, kt % 2, :], t[:, :],
                                         c_b[:, :])
                return z8s

            def ff_block(xin, w_d, bt, w2p_d, b2tt):
                """x + 0.5*ff(LN(x)); fp8 DoubleRow matmuls; returns new
                residual tiles.  Weights pre-scaled 2^S1 / 2^S2 on host."""
                r_b, c_b = layer_norm_rc(xin)
                z8s = ln_apply8(xin, r_b, c_b)
                # h = silu(z @ w1 * 2^-S1 + b1) directly on ACT, fp8 out
                w1ts = []
                for q in range(2):
                    wt = sb.tile([P, 2, FF], FP8, tag="w8", bufs=3)
                    nc.sync.dma_start(wt[:, :, :], w_d[q])
                    w1ts.append(wt)
                h8s = [sb.tile([P, FT // 2, N], FP8, tag="h8", bufs=2,
                               name=f"h8_{i}") for i in range(2)]
                for mt in range(FT):
                    ph = psp.tile([P, N], F32, tag="acc", bufs=4)
                    for q in range(2):
                        nc.tensor.matmul(ph[:, :],
                                         w1ts[q][:, :, mt * P:(mt + 1) * P],
                                         z8s[q][:, :, :],
                                         start=(q == 0), stop=(q == 1),
                                         perf_mode=DR)
                    nc.scalar.activation(h8s[mt // 8][:, mt % 8, :], ph[:, :],
                                         AF.Silu, bias=bt[:, mt:mt + 1],
                                         scale=S1INV)
                # y = h @ w2 * 2^-S2, pair-outer with 4 psum accumulators
                pys = [psp.tile([P, N], F32, tag="acc", bufs=4, name=f"pys{i}") for i in range(DT)]
                for r in range(FT // 2):
                    wt = sb.tile([P, 2, DIM], FP8, tag="w8s", bufs=10)
                    nc.sync.dma_start(wt[:, :, :], w2p_d[r])
                    rhs = h8s[(2 * r) // 8][:, (2 * r) % 8:(2 * r) % 8 + 2, :]
                    for mt in range(DT):
                        nc.tensor.matmul(pys[mt][:, :],
                                         wt[:, :, mt * P:(mt + 1) * P], rhs,
                                         start=(r == 0), stop=(r == FT // 2 - 1),
                                         perf_mode=DR)
                xo = []
                for mt in range(DT):
                    tb = sb.tile([P, N], F32, tag="lnt", bufs=2)
                    nc.scalar.activation(tb[:, :], pys[mt][:, :], AF.Identity,
                                         bias=b2tt[:, mt:mt + 1], scale=S2INV)
                    t = sb.tile([P, N], F32R, tag="x", bufs=6)
                    nc.vector.tensor_add(t[:, :], tb[:, :], xin[mt][:, :])
                    xo.append(t)
                return xo

            # ================= ff1 =================
            x1 = ff_block(xs, w1_d, b1t, w2_d, b2t)

            # ================= attention =================
            r_b, c_b = layer_norm_rc(x1)
            zs = ln_apply(x1, r_b, c_b)

            def proj_qk(w_dram, bias_t, tag):
                wts = []
                for kt in range(DT):
                    wt = sb.tile([P, INNER], F32R, tag="wsm", bufs=4)
                    nc.sync.dma_start(wt[:, :], w_dram[kt * P:(kt + 1) * P, :])
                    wts.append(wt)
                outs = []
                for mt in range(DT):
                    pq = psp.tile([P, N], F32, tag="mm", bufs=2)
                    for kt in range(DT):
                        nc.tensor.matmul(pq[:, :], r32(wts[kt][:, mt * P:(mt + 1) * P]),
                                         r32(zs[kt][:, :]),
                                         start=(kt == 0), stop=(kt == DT - 1))
                    qt = sb.tile([P, N], F32R, tag=tag, bufs=4)
                    nc.vector.tensor_scalar(out=qt[:, :], in0=pq[:, :],
                                            scalar1=bias_t[:, mt:mt + 1], scalar2=None,
                                            op0=AL.add)
                    outs.append(qt)
                return outs

            qTs = proj_qk(wq_d, bqt, "qT")
            kTs = proj_qk(wk_d, bkt, "kT")

            # v in time-major layout with a trailing ones column per head
            wvts = []
            for kt in range(DT):
                wt = sb.tile([P, INNER], F32R, tag="wsm", bufs=4)
                nc.sync.dma_start(wt[:, :], wv_d[kt * P:(kt + 1) * P, :])
                wvts.append(wt)
            vext = []
            for jt in range(DT):
                pv = psp.tile([P, N], F32, tag="mm", bufs=2)
                for kt in range(DT):
                    nc.tensor.matmul(pv[:, :], r32(zs[kt][:, jt * P:(jt + 1) * P]),
                                     r32(wvts[kt][:, :]),
                                     start=(kt == 0), stop=(kt == DT - 1))
                vx = sb.tile([P, H * 2 * DH], F32R, tag="vext", bufs=4)
                vw = vx[:, 0:H * 2 * DH].rearrange("p (h c) -> p h c", c=2 * DH)
                nc.vector.scalar_tensor_tensor(
                    vw[:, :, 0:DH],
                    pv[:, :].rearrange("p (h d) -> p h d", h=H), 1.0,
                    bvt[:, :].rearrange("p (h d) -> p h d", h=H),
                    AL.mult, AL.add)
                nc.vector.tensor_copy(
                    vw[:, :, DH:2 * DH],
                    ones_full[:, None, 0:DH].broadcast_to([P, H, DH]))
                vext.append(vx)

            # qr bounce (bf16 scratch) + scores, software-pipelined two heads
            # deep: while head h computes scores, head h+2's qr windows are
            # written (Scalar/GpSimd queues) and head h+1's rel tiles stream
            # back (Sync queue).
            def emit_qr(h):
                hb = (h % 2) * DH
                for it in range(DT):
                    lq = qTs[h // 2][hb:hb + DH, it * P:(it + 1) * P]
                    cr0 = 3 * P - P * it
                    pq1 = psp.tile([P, 320], F32, tag="acc", bufs=4)
                    nc.tensor.matmul(pq1[:, :], r32(lq),
                                     r32(relT[hb:hb + DH, cr0:cr0 + 320]),
                                     start=True, stop=True)
                    pq2 = psp.tile([P, 320], F32, tag="acc", bufs=4)
                    nc.tensor.matmul(pq2[:, :], r32(lq),
                                     r32(relT[hb:hb + DH, cr0 + 320:cr0 + 640]),
                                     start=True, stop=True)
                    qt = sb.tile([P, 640], FP16, tag="qt", bufs=3)
                    nc.vector.tensor_copy(qt[:, 0:320], pq1[:, :])
                    nc.vector.tensor_copy(qt[:, 320:640], pq2[:, :])
                    nc.gpsimd.dma_start(
                        qr_d[h, it * P:(it + 1) * P, cr0:cr0 + 640], qt[:, :])

            def emit_rel_reads(h):
                """One XBAR-transposing DMA per jt: rel_T[j, i] tiles direct
                from the scratch (globally qr[i, i-j+512] = addr i*1024 + j).
                Issue cost ~1.2us each, so spread 3:1 over Sync and Scalar."""
                rels = []
                for jt in range(DT):
                    rel = sb.tile([P, N], FP16, tag="rel", bufs=12)
                    src = bass.AP(qr_d, h * N * QRW + 4 * P + jt * P,
                                  [[QRW - 1, N], [1, P]])
                    eng = nc.scalar if jt == 3 else nc.sync
                    eng.dma_start_transpose(rel[:, :], src)
                    rels.append(rel)
                return rels

            oTs = [sb.tile([P, N], F32R, tag="oT", bufs=4, name=f"oTs{i}") for i in range(DT)]
            emit_qr(0)
            emit_qr(1)
            rel_q = [emit_rel_reads(0)]
            for h in range(H):
                hb = (h % 2) * DH
                rels = rel_q.pop(0)
                if h + 2 < H:
                    emit_qr(h + 2)
                if h + 1 < H:
                    rel_q.append(emit_rel_reads(h + 1))
                exps = []
                for jt in range(DT):
                    pss = psp.tile([P, N], F32, tag="acc", bufs=4)
                    nc.tensor.matmul(pss[:, :],
                                     r32(kTs[h // 2][hb:hb + DH, jt * P:(jt + 1) * P]),
                                     r32(qTs[h // 2][hb:hb + DH, :]),
                                     start=True, stop=True)
                    epre = sb.tile([P, N], F32, tag="tmp", bufs=3)
                    nc.vector.tensor_add(epre[:, :], pss[:, :], rels[jt][:, :])
                    e = sb.tile([P, N], F32R, tag="exp", bufs=5)
                    nc.scalar.activation(e[:, :], epre[:, :], AF.Exp)
                    exps.append(e)
                po = psp.tile([P, N], F32, tag="mm", bufs=2)
                for jt in range(DT):
                    nc.tensor.matmul(po[:, :],
                                     r32(vext[jt][:, h * 2 * DH:(h + 1) * 2 * DH]),
                                     r32(exps[jt][:, :]),
                                     start=(jt == 0), stop=(jt == DT - 1))
                lnd = sb.tile([DH, N], F32, tag="dwt", bufs=2)
                nc.scalar.activation(lnd[:, :], po[DH:2 * DH, :], AF.Ln)
                rb = sb.tile([DH, N], F32, tag="dwt", bufs=2)
                nc.scalar.activation(rb[:, :], lnd[:, :], AF.Exp, scale=-1.0)
                nc.vector.tensor_mul(oTs[h // 2][hb:hb + DH, :], po[0:DH, :],
                                     rb[:, :])

            # out-projection + residual
            wots = []
            for kt in range(DT):
                wt = sb.tile([P, DIM], F32R, tag="wsm", bufs=4)
                nc.sync.dma_start(wt[:, :], wo_d[kt * P:(kt + 1) * P, :])
                wots.append(wt)
            pas = [psp.tile([P, N], F32, tag="acc", bufs=4, name=f"pas{i}") for i in range(DT)]
            for kt in range(DT):
                for mt in range(DT):
                    nc.tensor.matmul(pas[mt][:, :], r32(wots[kt][:, mt * P:(mt + 1) * P]),
                                     r32(oTs[kt][:, :]),
                                     start=(kt == 0), stop=(kt == DT - 1))
            x2 = []
            for mt in range(DT):
                t = sb.tile([P, N], F32R, tag="x", bufs=6)
                nc.vector.scalar_tensor_tensor(t[:, :], pas[mt][:, :],
                                               bot[:, mt:mt + 1], x1[mt][:, :],
                                               AL.add, AL.add)
                x2.append(t)

            # ================= conv module =================
            # conv1 (fp8 DoubleRow): cast x2 to fp8, natural [a | g] col order
            x28s = [sb.tile([P, 2, N], FP8, tag="z8", bufs=4,
                            name=f"x28_{i}") for i in range(2)]
            for kt in range(DT):
                nc.vector.tensor_copy(x28s[kt // 2][:, kt % 2, :], x2[kt][:, :])
            c1ts = []
            for q in range(2):
                wt = sb.tile([P, 2, 2 * CIN], FP8, tag="w8", bufs=3)
                nc.sync.dma_start(wt[:, :, :], c1_d[q])
                c1ts.append(wt)
            GW = PAD + N + 2  # 544: causal pad + time + pad-to-32-taps slack
            glus = []
            for ct in range(CT):
                pa = psp.tile([P, N], F32, tag="acc", bufs=4)
                pg = psp.tile([P, N], F32, tag="acc", bufs=4)
                for q in range(2):
                    nc.tensor.matmul(pa[:, :],
                                     c1ts[q][:, :, ct * P:(ct + 1) * P],
                                     x28s[q][:, :, :],
                                     start=(q == 0), stop=(q == 1), perf_mode=DR)
                for q in range(2):
                    nc.tensor.matmul(pg[:, :],
                                     c1ts[q][:, :, CIN + ct * P:CIN + (ct + 1) * P],
                                     x28s[q][:, :, :],
                                     start=(q == 0), stop=(q == 1), perf_mode=DR)
                sig = sb.tile([P, N], F32, tag="tmp", bufs=3)
                nc.scalar.activation(sig[:, :], pg[:, :], AF.Sigmoid,
                                     bias=c1gt[:, ct:ct + 1], scale=S1INV)
                ya = sb.tile([P, N], F32, tag="tmp", bufs=3)
                nc.scalar.activation(ya[:, :], pa[:, :], AF.Identity,
                                     bias=c1at[:, ct:ct + 1], scale=S1INV)
                glu = sb.tile([P, GW], FP8, tag="glu8", bufs=8)
                nc.vector.memset(glu[:, 0:PAD], 0.0)
                nc.vector.memset(glu[:, PAD + N:GW], 0.0)
                nc.vector.tensor_mul(glu[:, PAD:PAD + N], ya[:, :], sig[:, :])
                glus.append(glu)

            # depthwise conv: 16 DoubleRow tap-pair matmuls per channel block
            # (tap 31 is a zero diagonal).  Moving operand is an overlapping
            # [P, 2, N] view of the padded glu tile (stride-1 tap shift).
            hc8s = [sb.tile([P, CT // 2, N], FP8, tag="h8", bufs=2,
                            name=f"hc8_{i}") for i in range(2)]
            for ct in range(CT):
                dg = sb.tile([P, KP, P], FP8, tag="dg8", bufs=3)
                nc.sync.dma_start(dg[:, :, :], dwd_d[ct])
                pd = psp.tile([P, N], F32, tag="mm", bufs=2)
                gbase = glus[ct][:, 0:GW]
                for r in range(KP // 2):
                    rhs = bass.AP(gbase.tensor, gbase.offset + 2 * r,
                                  [[GW, P], [1, 2], [1, N]])
                    nc.tensor.matmul(pd[:, :], dg[:, 2 * r:2 * r + 2, :], rhs,
                                     start=(r == 0), stop=(r == KP // 2 - 1),
                                     perf_mode=DR)
                sig = sb.tile([P, N], F32, tag="dwt", bufs=2)
                nc.scalar.activation(sig[:, :], pd[:, :], AF.Sigmoid,
                                     bias=bntt[:, ct:ct + 1], scale=bnst[:, ct:ct + 1])
                u = sb.tile([P, N], F32, tag="dwt", bufs=2)
                nc.vector.tensor_scalar(out=u[:, :], in0=pd[:, :],
                                        scalar1=bnst[:, ct:ct + 1],
                                        scalar2=bntt[:, ct:ct + 1],
                                        op0=AL.mult, op1=AL.add)
                nc.vector.tensor_mul(hc8s[ct // 4][:, ct % 4, :], u[:, :],
                                     sig[:, :])

            # conv2 (fp8 DoubleRow) + residual
            c2ts = []
            for q in range(CT // 2):
                wt = sb.tile([P, 2, DIM], FP8, tag="w8s", bufs=10)
                nc.sync.dma_start(wt[:, :, :], c2_d[q])
                c2ts.append(wt)
            pcs = [psp.tile([P, N], F32, tag="acc", bufs=4, name=f"pcs{i}") for i in range(DT)]
            for q in range(CT // 2):
                rhs = hc8s[q // 2][:, (2 * q) % 4:(2 * q) % 4 + 2, :]
                for mt in range(DT):
                    nc.tensor.matmul(pcs[mt][:, :],
                                     c2ts[q][:, :, mt * P:(mt + 1) * P],
                                     rhs,
                                     start=(q == 0), stop=(q == CT // 2 - 1),
                                     perf_mode=DR)
            x3 = []
            for mt in range(DT):
                tb = sb.tile([P, N], F32, tag="lnt", bufs=2)
                nc.scalar.activation(tb[:, :], pcs[mt][:, :], AF.Identity,
                                     bias=c2bt[:, mt:mt + 1], scale=S1INV)
                t = sb.tile([P, N], F32R, tag="x", bufs=6)
                nc.vector.tensor_add(t[:, :], tb[:, :], x2[mt][:, :])
                x3.append(t)

            # ================= ff2 =================
            x4 = ff_block(x3, w3_d, b3t, w4_d, b4t)

            # ================= post-LN =================
            r_b, c_b = layer_norm_rc(x4)
            for mt in range(DT):
                t = sb.tile([P, N], F32, tag="lnt", bufs=2)
                nc.vector.tensor_mul(t[:, :], x4[mt][:, :], r_b[:, :])
                t2 = sb.tile([P, N], F32, tag="lnt2", bufs=1)
                nc.vector.tensor_add(t2[:, :], t[:, :], c_b[:, :])
                ot = sb.tile([P, N], F32, tag="outt", bufs=2)
                nc.vector.tensor_scalar(out=ot[:, :], in0=t2[:, :],
                                        scalar1=pngt[:, mt:mt + 1],
                                        scalar2=pnbt[:, mt:mt + 1],
                                        op0=AL.mult, op1=AL.add)
                eng = (nc.scalar, nc.sync, nc.gpsimd, nc.scalar)[mt]
                eng.dma_start(outT_d[mt * P:(mt + 1) * P, :], ot[:, :])

    if split_waits:
        _split_matmul_waits(nc, mybir)
    return nc


def _split_matmul_waits(nc, mybir):
    """This walrus build rejects engine instructions carrying more than one
    sync wait; hoist the extras onto EventSemaphore instructions on the same
    engine queue right before the instruction."""
    fn = nc.m.functions[0]
    ctr = 0
    for blk in fn.blocks:
        out = []
        changed = False
        for ins in blk.instructions:
            si = ins.sync_info
            if (si is not None and si.on_wait and len(si.on_wait) > 1
                    and not isinstance(ins, (mybir.InstEventSemaphore,
                                             mybir.InstNoOp))):
                waits = list(si.on_wait)
                for w in waits[:-1]:
                    ev = mybir.InstNoOp(
                        name=f"EVW-{ctr}", ins=[], outs=[],
                        sync_info=mybir.SyncInfo(on_wait=[w], on_update=[]))
                    ev.engine = ins.engine
                    ctr += 1
                    out.append(ev)
                ins.sync_info = mybir.SyncInfo(
                    on_wait=[waits[-1]], on_update=list(si.on_update or []))
                changed = True
            out.append(ins)
        if changed:
            blk.instructions = out


def prep_inputs(inputs):
    """Host-side preprocessing: fold LN affines / scales / biases into weights."""
    import ml_dtypes

    f = np.float32
    f8 = ml_dtypes.float8_e4m3
    ii = {k: np.asarray(v, dtype=f) for k, v in inputs.items()}

    def colmaj(b, nb):
        return np.ascontiguousarray(b.astype(f).reshape(nb, P).T)

    def pack_pairs(w, spow):
        """[K, M] weight -> [K//256, P, 2, M] fp8, scaled by 2^spow."""
        K, M = w.shape
        t = (w * (2.0 ** spow)).reshape(K // (2 * P), 2, P, M).transpose(0, 2, 1, 3)
        return np.ascontiguousarray(t.astype(f8))

    g1, be1 = ii["ff1_ln_g"], ii["ff1_ln_b"]
    w1 = pack_pairs(g1[:, None] * ii["ff1_w1"], S1POW)
    b1 = colmaj(be1 @ ii["ff1_w1"] + ii["ff1_b1"], FT)
    w2 = pack_pairs(0.5 * ii["ff1_w2"], S2POW)
    b2 = colmaj(0.5 * ii["ff1_b2"], DT)

    ag, ab = ii["attn_ln_g"], ii["attn_ln_b"]
    sc = DH ** -0.5
    wq = np.ascontiguousarray(ag[:, None] * ii["wq"] * sc)
    bq = colmaj((ab @ ii["wq"] + ii["bq"]) * sc, DT)
    wkv, bkv = ii["wkv"], ii["bkv"]
    wk = np.ascontiguousarray(ag[:, None] * wkv[:, :INNER])
    bk = colmaj(ab @ wkv[:, :INNER] + bkv[:INNER], DT)
    wv = np.ascontiguousarray(ag[:, None] * wkv[:, INNER:])
    bv = np.ascontiguousarray(np.broadcast_to(
        ab @ wkv[:, INNER:] + bkv[INNER:], (P, INNER)))
    wo = np.ascontiguousarray(ii["wo"])
    bo = colmaj(ii["bo"], DT)
    # relT rows: head feature d lives at partition (h%2)*64 + d -> duplicate rows
    rT = ii["rel_emb"].T[:, ::-1]  # [64, 1025] column-reversed
    relT = np.ascontiguousarray(np.concatenate([rT, rT], axis=0))  # [128, 1025]

    # conv1 in natural [a(0:CIN) | g(CIN:2CIN)] column order
    c1 = pack_pairs(ii["conv1_w"], S1POW)
    c1b = ii["conv1_b"]
    c1a = colmaj(c1b[:CIN], CT)
    c1g = colmaj(c1b[CIN:], CT)
    # depthwise taps as diagonal pair blocks [CT, P, 32, P] (tap 31 zero)
    KP = KW + 1
    dwd = np.zeros((CT, P, KP, P), dtype=f)
    wr = ii["dw_w"].reshape(CT, P, KW) * (2.0 ** SDPOW)
    pp = np.arange(P)
    for ct in range(CT):
        for k in range(KW):
            dwd[ct, pp, k, pp] = wr[ct, :, k]
    dwd = np.ascontiguousarray(dwd.astype(f8))
    inv = 1.0 / np.sqrt(ii["bn_var"] + EPS)
    s = inv * ii["bn_g"]
    t = ii["bn_b"] - ii["bn_mean"] * s
    bns = colmaj(s * (2.0 ** -SDPOW), CT)
    bnt = colmaj(t + s * ii["dw_b"], CT)
    c2 = pack_pairs(ii["conv2_w"], S1POW)
    c2b = colmaj(ii["conv2_b"], DT)

    g3, be3 = ii["ff2_ln_g"], ii["ff2_ln_b"]
    w3 = pack_pairs(g3[:, None] * ii["ff2_w1"], S1POW)
    b3 = colmaj(be3 @ ii["ff2_w1"] + ii["ff2_b1"], FT)
    w4 = pack_pairs(0.5 * ii["ff2_w2"], S2POW)
    b4 = colmaj(0.5 * ii["ff2_b2"], DT)

    png = colmaj(ii["pn_g"], DT)
    pnb = colmaj(ii["pn_b"], DT)

    # pack the small [P, k] constants into one tensor; column map mirrors build()
    pk = np.concatenate([b1, b2, bq, bk, bo, c1a, c1g, bns, bnt, c2b,
                         b3, b4, png, pnb], axis=1)
    assert pk.shape == (P, 96), pk.shape
    shared = dict(w1p=w1, w2p=w2, wq=wq, wk=wk,
                  wv=wv, bvb=bv, wo=wo, relT=relT, c1p=c1,
                  dwp=dwd, c2p=c2,
                  w3p=w3, w4p=w4,
                  cstpack=np.ascontiguousarray(pk),
                  antid=np.ascontiguousarray(np.eye(P, dtype=f)),
                  onesf=np.ones((P, P), dtype=f),
                  onesm=np.full((P, P), 1.0 / DIM, dtype=f))
    x = ii["x"]
    in_maps = []
    for b in range(NCORES):
        m = dict(shared)
        m["xT"] = np.ascontiguousarray(x[b].T)
        in_maps.append(m)
    return in_maps


_BUILT = None


def run(inputs, trace=False):
    global _BUILT
    from concourse import bass_utils

    in_maps = prep_inputs(inputs)
    if _BUILT is None:
        _BUILT = build()
    res = bass_utils.run_bass_kernel_spmd(
        _BUILT, in_maps, core_ids=list(range(NCORES)), trace=trace)
    out = np.stack([np.asarray(r["outT"]).T for r in res.results])
    return np.ascontiguousarray(out.astype(np.float32)), res


def kernel(**inputs):
    out, _ = run(inputs, trace=False)
    return out



# revision 59
# speedup vs baseline: 1.0672x; 1.0632x over previous
"""Trainium2 Bass kernel: Conformer block (B=8, N=512, DIM=512, H=8, DH=64, FF=2048, CIN=1024, K=31).

Sharding: pure data-parallel over batch — each of the 8 NeuronCores processes one
batch item with the full weight set (no collectives).

Layout: activations are kept FEATURE-major ([feature, time] = x.T) on chip so that
chained matmuls need no transposes (weights stay in natural [din, dout] layout as
the stationary operand).  LayerNorm reductions over features become ones-vector
matmuls on the PE; per-time-step affine factors are broadcast across partitions
with a GPSIMD partition_broadcast.

Relative-position attention uses the shift-gather trick: qr = q @ rel_emb.T is
bounced through an internal DRAM scratch and read back with a strided
(stride = row+1, step -1) access pattern so that rel[j, i] = qr[i, i-j+512]
lands directly as the transposed score tile.  Scores are computed transposed
(dots_T[j, i]) so softmax runs over the partition axis: exp on ACT, the
denominator via a ones-column fused into the attn@v matmul, and the final
normalization as a broadcasted multiply.

The causal depthwise conv runs on the PE as 31 PSUM-accumulated matmuls per
128-channel block against diagonal stationary matrices; the diagonals are
(re)written with a single strided DMA per block (dst step = row+1).

Matmuls use float32r (1 cycle/row for N>=256); the FFN second matmul and the
depthwise conv run in bf16.
"""

import sys

for _p in ("/opt/trn_rl_repo", "/root/.axon_site/_ro/trn_rl_repo"):
    if _p not in sys.path:
        sys.path.insert(0, _p)

import numpy as np

B, N, DIM, H, DH, MULT, EXP, KW, MAXP = 8, 512, 512, 8, 64, 4, 2, 31, 512
INNER = H * DH
FF = DIM * MULT
CIN = DIM * EXP
EPS = 1e-5
P = 128
DT = DIM // P      # 4  feature tiles of the residual stream
FT = FF // P       # 16 ff hidden tiles
CT = CIN // P      # 8  conv channel tiles
NCORES = 8
PAD = KW - 1       # 30 causal pad
S1POW = 10         # fp8 pre-scale exponent: w1/w3/c1/c2
S2POW = 11         # fp8 pre-scale exponent: w2/w4 (include the 0.5)
SDPOW = 9          # fp8 pre-scale exponent: depthwise taps (folded into bns)


def build(split_waits=True):
    """Build the single-core Bass module (SPMD: same NEFF on all 8 cores)."""
    import concourse.bass as bass
    import concourse.mybir as mybir
    import concourse.tile as tile

    F32 = mybir.dt.float32
    F32R = mybir.dt.float32r
    BF16 = mybir.dt.bfloat16
    AF = mybir.ActivationFunctionType
    AL = mybir.AluOpType

    nc = bass.Bass()

    # ---------------- I/O ----------------
    FP8 = mybir.dt.float8e4
    FP16 = mybir.dt.float16
    KP = KW + 1  # dw taps padded to 32 for even DoubleRow pairing

    xT_d = nc.dram_tensor("xT", [DIM, N], F32R, kind="ExternalInput")
    # fp8 DoubleRow-packed weights: [n_pairs, P, 2, cols], scaled by 2^S*
    w1_d = nc.dram_tensor("w1p", [2, P, 2, FF], FP8, kind="ExternalInput")
    w2_d = nc.dram_tensor("w2p", [FT // 2, P, 2, DIM], FP8, kind="ExternalInput")
    wq_d = nc.dram_tensor("wq", [DIM, INNER], F32R, kind="ExternalInput")
    wk_d = nc.dram_tensor("wk", [DIM, INNER], F32R, kind="ExternalInput")
    wv_d = nc.dram_tensor("wv", [DIM, INNER], F32R, kind="ExternalInput")
    bv_d = nc.dram_tensor("bvb", [P, INNER], F32R, kind="ExternalInput")
    wo_d = nc.dram_tensor("wop", [2, P, 2, DIM], FP8, kind="ExternalInput")
    relT_d = nc.dram_tensor("relT", [P, 2 * MAXP + 1], F32R, kind="ExternalInput")
    c1_d = nc.dram_tensor("c1p", [2, P, 2, 2 * CIN], FP8, kind="ExternalInput")
    dwd_d = nc.dram_tensor("dwp", [CT, P, KP, P], FP8, kind="ExternalInput")
    c2_d = nc.dram_tensor("c2p", [CT // 2, P, 2, DIM], FP8, kind="ExternalInput")
    w3_d = nc.dram_tensor("w3p", [2, P, 2, FF], FP8, kind="ExternalInput")
    w4_d = nc.dram_tensor("w4p", [FT // 2, P, 2, DIM], FP8, kind="ExternalInput")
    antid_d = nc.dram_tensor("antid", [P, P], F32R, kind="ExternalInput")
    onesf_d = nc.dram_tensor("onesf", [P, P], F32R, kind="ExternalInput")
    onesm_d = nc.dram_tensor("onesm", [P, P], F32R, kind="ExternalInput")
    # packed small per-tile biases/affines: see prep_inputs for column map
    NPK = 96
    pk_d = nc.dram_tensor("cstpack", [P, NPK], F32, kind="ExternalInput")

    outT_d = nc.dram_tensor("outT", [DIM, N], F32, kind="ExternalOutput")

    QRW = 2 * MAXP + 1  # 1025 scratch row width
    qr_d = nc.dram_tensor("qr_scratch", [H, N, QRW], FP16, kind="Internal")

    def r32(ap):
        return ap.bitcast(F32R)

    DR = mybir.MatmulPerfMode.DoubleRow
    S1INV = 2.0 ** -S1POW   # w1/w3/c1/c2 pre-scale compensation
    S2INV = 2.0 ** -S2POW   # w2/w4 pre-scale compensation

    with tile.TileContext(nc) as tc:
        with (
            nc.allow_low_precision(reason="fp32r/bf16 matmul feeds"),
            tc.tile_pool(name="cst", bufs=1) as cst,
            tc.tile_pool(name="sb", bufs=2) as sb,
            tc.tile_pool(name="ps", bufs=2, space="PSUM") as psp,
        ):

            # ---------------- x + constants (x first: LN stats need only x) ---
            ones_full = cst.tile([P, P], F32R, tag="ones_full")
            nc.sync.dma_start(ones_full[:, :], onesf_d[:, :])
            ones_mean = cst.tile([P, P], F32R, tag="ones_mean")
            nc.sync.dma_start(ones_mean[:, :], onesm_d[:, :])
            xs = []
            for mt in range(DT):
                xt = sb.tile([P, N], F32R, tag="x", bufs=6)
                nc.sync.dma_start(xt[:, :], xT_d[mt * P:(mt + 1) * P, :])
                xs.append(xt)
            pkt = cst.tile([P, NPK], F32, tag="cstpack")
            nc.sync.dma_start(pkt[:, :], pk_d[:, :])
            b1t = pkt[:, 0:16]
            b2t = pkt[:, 16:20]
            bqt = pkt[:, 20:24]
            bkt = pkt[:, 24:28]
            bot = pkt[:, 28:32]
            c1at = pkt[:, 32:40]
            c1gt = pkt[:, 40:48]
            bnst = pkt[:, 48:56]
            bntt = pkt[:, 56:64]
            c2bt = pkt[:, 64:68]
            b3t = pkt[:, 68:84]
            b4t = pkt[:, 84:88]
            pngt = pkt[:, 88:92]
            pnbt = pkt[:, 92:96]
            relT = cst.tile([P, QRW], F32R, tag="relT")
            nc.sync.dma_start(relT[:, :], relT_d[:, :])
            bvt = cst.tile([P, INNER], F32R, tag="bvt")
            nc.sync.dma_start(bvt[:, :], bv_d[:, :])

            # ---------------- helpers ----------------
            def layer_norm_rc(xin):
                """Stats of LN over the partition (feature) axis.

                Returns r_b, c_b [128, 512] tiles with z = x*r_b + c_b.
                The 1/DIM is folded into the ones_mean stationary; squares run
                on the otherwise-idle GpSimd engine."""
                ps_mean = psp.tile([P, N], F32, tag="mm", bufs=2)
                for kt in range(DT):
                    nc.tensor.matmul(ps_mean[:, :], ones_mean[:, :], xin[kt][:, :],
                                     start=(kt == 0), stop=(kt == DT - 1))
                ps_sq = psp.tile([P, N], F32, tag="mm", bufs=2)
                for kt in range(DT):
                    xsq = sb.tile([P, N], F32R, tag="tmp", bufs=3)
                    nc.scalar.square(xsq[:, :], xin[kt][:, :])
                    nc.tensor.matmul(ps_sq[:, :], ones_mean[:, :], xsq[:, :],
                                     start=(kt == 0), stop=(kt == DT - 1))
                nm2 = sb.tile([P, N], F32, tag="tmp", bufs=3)
                nc.scalar.activation(nm2[:, :], ps_mean[:, :], AF.Square)
                veps = sb.tile([P, N], F32, tag="tmp", bufs=3)
                nc.vector.scalar_tensor_tensor(veps[:, :], ps_sq[:, :], EPS,
                                               nm2[:, :], AL.add, AL.subtract)
                lnv = sb.tile([P, N], F32, tag="tmp", bufs=3)
                nc.scalar.activation(lnv[:, :], veps[:, :], AF.Ln)
                r_b = sb.tile([P, N], F32, tag="r_b", bufs=2)
                nc.scalar.activation(r_b[:, :], lnv[:, :], AF.Exp, scale=-0.5)
                c_b = sb.tile([P, N], F32, tag="c_b", bufs=2)
                nc.vector.scalar_tensor_tensor(c_b[:, :], ps_mean[:, :], -1.0,
                                               r_b[:, :], AL.mult, AL.mult)
                return r_b, c_b

            def ln_apply(xin, r_b, c_b):
                zs = []
                for kt in range(DT):
                    t = sb.tile([P, N], F32, tag="lnt", bufs=2)
                    nc.vector.tensor_mul(t[:, :], xin[kt][:, :], r_b[:, :])
                    z = sb.tile([P, N], F32R, tag="z", bufs=4)
                    nc.vector.tensor_add(z[:, :], t[:, :], c_b[:, :])
                    zs.append(z)
                return zs

            def ln_apply8(xin, r_b, c_b):
                """LN apply straight into fp8 pair tiles (mm1 moving); two
                tiles so the first DoubleRow pair can start after 2 writes."""
                z8s = [sb.tile([P, 2, N], FP8, tag="z8", bufs=4,
                               name=f"z8_{i}") for i in range(2)]
                for kt in range(DT):
                    t = sb.tile([P, N], F32, tag="lnt", bufs=2)
                    nc.vector.tensor_mul(t[:, :], xin[kt][:, :], r_b[:, :])
                    nc.vector.tensor_add(z8s[kt // 2][:, kt % 2, :], t[:, :],
                                         c_b[:, :])
                return z8s

            def ff_block(xin, w_d, bt, w2p_d, b2tt):
                """x + 0.5*ff(LN(x)); fp8 DoubleRow matmuls; returns new
                residual tiles.  Weights pre-scaled 2^S1 / 2^S2 on host."""
                r_b, c_b = layer_norm_rc(xin)
                z8s = ln_apply8(xin, r_b, c_b)
                # h = silu(z @ w1 * 2^-S1 + b1) directly on ACT, fp8 out
                w1ts = []
                for q in range(2):
                    wt = sb.tile([P, 2, FF], FP8, tag="w8", bufs=3)
                    nc.sync.dma_start(wt[:, :, :], w_d[q])
                    w1ts.append(wt)
                h8s = [sb.tile([P, FT // 2, N], FP8, tag="h8", bufs=2,
                               name=f"h8_{i}") for i in range(2)]
                for mt in range(FT):
                    ph = psp.tile([P, N], F32, tag="acc", bufs=4)
                    for q in range(2):
                        nc.tensor.matmul(ph[:, :],
                                         w1ts[q][:, :, mt * P:(mt + 1) * P],
                                         z8s[q][:, :, :],
                                         start=(q == 0), stop=(q == 1),
                                         perf_mode=DR)
                    nc.scalar.activation(h8s[mt // 8][:, mt % 8, :], ph[:, :],
                                         AF.Silu, bias=bt[:, mt:mt + 1],
                                         scale=S1INV)
                # y = h @ w2 * 2^-S2, pair-outer with 4 psum accumulators
                pys = [psp.tile([P, N], F32, tag="acc", bufs=4, name=f"pys{i}") for i in range(DT)]
                for r in range(FT // 2):
                    wt = sb.tile([P, 2, DIM], FP8, tag="w8s", bufs=10)
                    nc.sync.dma_start(wt[:, :, :], w2p_d[r])
                    rhs = h8s[(2 * r) // 8][:, (2 * r) % 8:(2 * r) % 8 + 2, :]
                    for mt in range(DT):
                        nc.tensor.matmul(pys[mt][:, :],
                                         wt[:, :, mt * P:(mt + 1) * P], rhs,
                                         start=(r == 0), stop=(r == FT // 2 - 1),
                                         perf_mode=DR)
                xo = []
                for mt in range(DT):
                    tb = sb.tile([P, N], F32, tag="lnt", bufs=2)
                    nc.scalar.activation(tb[:, :], pys[mt][:, :], AF.Identity,
                                         bias=b2tt[:, mt:mt + 1], scale=S2INV)
                    t = sb.tile([P, N], F32R, tag="x", bufs=6)
                    nc.vector.tensor_add(t[:, :], tb[:, :], xin[mt][:, :])
                    xo.append(t)
                return xo

            # ================= ff1 =================
            x1 = ff_block(xs, w1_d, b1t, w2_d, b2t)

            # ================= attention =================
            r_b, c_b = layer_norm_rc(x1)
            zs = ln_apply(x1, r_b, c_b)

            def proj_qk(w_dram, bias_t, tag):
                wts = []
                for kt in range(DT):
                    wt = sb.tile([P, INNER], F32R, tag="wsm", bufs=4)
                    nc.sync.dma_start(wt[:, :], w_dram[kt * P:(kt + 1) * P, :])
                    wts.append(wt)
                outs = []
                for mt in range(DT):
                    pq = psp.tile([P, N], F32, tag="mm", bufs=2)
                    for kt in range(DT):
                        nc.tensor.matmul(pq[:, :], r32(wts[kt][:, mt * P:(mt + 1) * P]),
                                         r32(zs[kt][:, :]),
                                         start=(kt == 0), stop=(kt == DT - 1))
                    qt = sb.tile([P, N], F32R, tag=tag, bufs=4)
                    nc.vector.tensor_scalar(out=qt[:, :], in0=pq[:, :],
                                            scalar1=bias_t[:, mt:mt + 1], scalar2=None,
                                            op0=AL.add)
                    outs.append(qt)
                return outs

            qTs = proj_qk(wq_d, bqt, "qT")
            kTs = proj_qk(wk_d, bkt, "kT")

            # v in time-major layout with a trailing ones column per head
            wvts = []
            for kt in range(DT):
                wt = sb.tile([P, INNER], F32R, tag="wsm", bufs=4)
                nc.sync.dma_start(wt[:, :], wv_d[kt * P:(kt + 1) * P, :])
                wvts.append(wt)
            vext8 = sb.tile([P, DT, H * 2 * DH], FP8, tag="vext", bufs=1)
            for jt in range(DT):
                pv = psp.tile([P, N], F32, tag="mm", bufs=2)
                for kt in range(DT):
                    nc.tensor.matmul(pv[:, :], r32(zs[kt][:, jt * P:(jt + 1) * P]),
                                     r32(wvts[kt][:, :]),
                                     start=(kt == 0), stop=(kt == DT - 1))
                vw = vext8[:, jt, :].rearrange("p (h c) -> p h c", c=2 * DH)
                nc.vector.scalar_tensor_tensor(
                    vw[:, :, 0:DH],
                    pv[:, :].rearrange("p (h d) -> p h d", h=H), 1.0,
                    bvt[:, :].rearrange("p (h d) -> p h d", h=H),
                    AL.mult, AL.add)
                nc.vector.tensor_copy(
                    vw[:, :, DH:2 * DH],
                    ones_full[:, None, 0:DH].broadcast_to([P, H, DH]))

            # qr bounce (bf16 scratch) + scores, software-pipelined two heads
            # deep: while head h computes scores, head h+2's qr windows are
            # written (Scalar/GpSimd queues) and head h+1's rel tiles stream
            # back (Sync queue).
            def emit_qr(h):
                hb = (h % 2) * DH
                for it in range(DT):
                    lq = qTs[h // 2][hb:hb + DH, it * P:(it + 1) * P]
                    cr0 = 3 * P - P * it
                    pq1 = psp.tile([P, 320], F32, tag="acc", bufs=4)
                    nc.tensor.matmul(pq1[:, :], r32(lq),
                                     r32(relT[hb:hb + DH, cr0:cr0 + 320]),
                                     start=True, stop=True)
                    pq2 = psp.tile([P, 320], F32, tag="acc", bufs=4)
                    nc.tensor.matmul(pq2[:, :], r32(lq),
                                     r32(relT[hb:hb + DH, cr0 + 320:cr0 + 640]),
                                     start=True, stop=True)
                    qt = sb.tile([P, 640], FP16, tag="qt", bufs=3)
                    nc.vector.tensor_copy(qt[:, 0:320], pq1[:, :])
                    nc.vector.tensor_copy(qt[:, 320:640], pq2[:, :])
                    nc.gpsimd.dma_start(
                        qr_d[h, it * P:(it + 1) * P, cr0:cr0 + 640], qt[:, :])

            def emit_rel_reads(h):
                """One XBAR-transposing DMA per jt: rel_T[j, i] tiles direct
                from the scratch (globally qr[i, i-j+512] = addr i*1024 + j).
                Issue cost ~1.2us each, so spread 3:1 over Sync and Scalar."""
                rels = []
                for jt in range(DT):
                    rel = sb.tile([P, N], FP16, tag="rel", bufs=12)
                    src = bass.AP(qr_d, h * N * QRW + 4 * P + jt * P,
                                  [[QRW - 1, N], [1, P]])
                    eng = nc.scalar if jt == 3 else nc.sync
                    eng.dma_start_transpose(rel[:, :], src)
                    rels.append(rel)
                return rels

            oT8 = sb.tile([P, DT, N], FP8, tag="oT", bufs=1)
            emit_qr(0)
            emit_qr(1)
            rel_q = [emit_rel_reads(0)]
            for h in range(H):
                hb = (h % 2) * DH
                rels = rel_q.pop(0)
                if h + 2 < H:
                    emit_qr(h + 2)
                if h + 1 < H:
                    rel_q.append(emit_rel_reads(h + 1))
                exps8 = sb.tile([P, DT, N], FP8, tag="exp", bufs=3)
                for jt in range(DT):
                    pss = psp.tile([P, N], F32, tag="acc", bufs=4)
                    nc.tensor.matmul(pss[:, :],
                                     r32(kTs[h // 2][hb:hb + DH, jt * P:(jt + 1) * P]),
                                     r32(qTs[h // 2][hb:hb + DH, :]),
                                     start=True, stop=True)
                    epre = sb.tile([P, N], F32, tag="tmp", bufs=3)
                    nc.vector.tensor_add(epre[:, :], pss[:, :], rels[jt][:, :])
                    nc.scalar.activation(exps8[:, jt, :], epre[:, :], AF.Exp)
                po = psp.tile([P, N], F32, tag="mm", bufs=2)
                for r in range(DT // 2):
                    nc.tensor.matmul(po[:, :],
                                     vext8[:, 2 * r:2 * r + 2,
                                           h * 2 * DH:(h + 1) * 2 * DH],
                                     exps8[:, 2 * r:2 * r + 2, :],
                                     start=(r == 0), stop=(r == DT // 2 - 1),
                                     perf_mode=DR)
                lnd = sb.tile([DH, N], F32, tag="dwt", bufs=2)
                nc.scalar.activation(lnd[:, :], po[DH:2 * DH, :], AF.Ln)
                rb = sb.tile([DH, N], F32, tag="dwt", bufs=2)
                nc.scalar.activation(rb[:, :], lnd[:, :], AF.Exp, scale=-1.0)
                nc.vector.tensor_mul(oT8[hb:hb + DH, h // 2, :], po[0:DH, :],
                                     rb[:, :])

            # out-projection (fp8 DoubleRow) + residual
            wots = []
            for q in range(2):
                wt = sb.tile([P, 2, DIM], FP8, tag="w8s", bufs=10)
                nc.sync.dma_start(wt[:, :, :], wo_d[q])
                wots.append(wt)
            pas = [psp.tile([P, N], F32, tag="acc", bufs=4, name=f"pas{i}") for i in range(DT)]
            for q in range(2):
                for mt in range(DT):
                    nc.tensor.matmul(pas[mt][:, :],
                                     wots[q][:, :, mt * P:(mt + 1) * P],
                                     oT8[:, 2 * q:2 * q + 2, :],
                                     start=(q == 0), stop=(q == 1),
                                     perf_mode=DR)
            x2 = []
            for mt in range(DT):
                tb = sb.tile([P, N], F32, tag="lnt", bufs=2)
                nc.scalar.activation(tb[:, :], pas[mt][:, :], AF.Identity,
                                     bias=bot[:, mt:mt + 1], scale=S1INV)
                t = sb.tile([P, N], F32R, tag="x", bufs=6)
                nc.vector.tensor_add(t[:, :], tb[:, :], x1[mt][:, :])
                x2.append(t)

            # ================= conv module =================
            # conv1 (fp8 DoubleRow): cast x2 to fp8, natural [a | g] col order
            x28s = [sb.tile([P, 2, N], FP8, tag="z8", bufs=4,
                            name=f"x28_{i}") for i in range(2)]
            for kt in range(DT):
                nc.vector.tensor_copy(x28s[kt // 2][:, kt % 2, :], x2[kt][:, :])
            c1ts = []
            for q in range(2):
                wt = sb.tile([P, 2, 2 * CIN], FP8, tag="w8", bufs=3)
                nc.sync.dma_start(wt[:, :, :], c1_d[q])
                c1ts.append(wt)
            GW = PAD + N + 2  # 544: causal pad + time + pad-to-32-taps slack
            glus = []
            for ct in range(CT):
                pa = psp.tile([P, N], F32, tag="acc", bufs=4)
                pg = psp.tile([P, N], F32, tag="acc", bufs=4)
                for q in range(2):
                    nc.tensor.matmul(pa[:, :],
                                     c1ts[q][:, :, ct * P:(ct + 1) * P],
                                     x28s[q][:, :, :],
                                     start=(q == 0), stop=(q == 1), perf_mode=DR)
                for q in range(2):
                    nc.tensor.matmul(pg[:, :],
                                     c1ts[q][:, :, CIN + ct * P:CIN + (ct + 1) * P],
                                     x28s[q][:, :, :],
                                     start=(q == 0), stop=(q == 1), perf_mode=DR)
                sig = sb.tile([P, N], F32, tag="tmp", bufs=3)
                nc.scalar.activation(sig[:, :], pg[:, :], AF.Sigmoid,
                                     bias=c1gt[:, ct:ct + 1], scale=S1INV)
                ya = sb.tile([P, N], F32, tag="tmp", bufs=3)
                nc.scalar.activation(ya[:, :], pa[:, :], AF.Identity,
                                     bias=c1at[:, ct:ct + 1], scale=S1INV)
                glu = sb.tile([P, GW], FP8, tag="glu8", bufs=8)
                nc.vector.memset(glu[:, 0:PAD], 0.0)
                nc.vector.memset(glu[:, PAD + N:GW], 0.0)
                nc.vector.tensor_mul(glu[:, PAD:PAD + N], ya[:, :], sig[:, :])
                glus.append(glu)

            # depthwise conv: 16 DoubleRow tap-pair matmuls per channel block
            # (tap 31 is a zero diagonal).  Moving operand is an overlapping
            # [P, 2, N] view of the padded glu tile (stride-1 tap shift).
            hc8s = [sb.tile([P, CT // 2, N], FP8, tag="h8", bufs=2,
                            name=f"hc8_{i}") for i in range(2)]
            for ct in range(CT):
                dg = sb.tile([P, KP, P], FP8, tag="dg8", bufs=3)
                nc.sync.dma_start(dg[:, :, :], dwd_d[ct])
                pd = psp.tile([P, N], F32, tag="mm", bufs=2)
                gbase = glus[ct][:, 0:GW]
                for r in range(KP // 2):
                    rhs = bass.AP(gbase.tensor, gbase.offset + 2 * r,
                                  [[GW, P], [1, 2], [1, N]])
                    nc.tensor.matmul(pd[:, :], dg[:, 2 * r:2 * r + 2, :], rhs,
                                     start=(r == 0), stop=(r == KP // 2 - 1),
                                     perf_mode=DR)
                sig = sb.tile([P, N], F32, tag="dwt", bufs=2)
                nc.scalar.activation(sig[:, :], pd[:, :], AF.Sigmoid,
                                     bias=bntt[:, ct:ct + 1], scale=bnst[:, ct:ct + 1])
                u = sb.tile([P, N], F32, tag="dwt", bufs=2)
                nc.vector.tensor_scalar(out=u[:, :], in0=pd[:, :],
                                        scalar1=bnst[:, ct:ct + 1],
                                        scalar2=bntt[:, ct:ct + 1],
                                        op0=AL.mult, op1=AL.add)
                nc.vector.tensor_mul(hc8s[ct // 4][:, ct % 4, :], u[:, :],
                                     sig[:, :])

            # conv2 (fp8 DoubleRow) + residual
            c2ts = []
            for q in range(CT // 2):
                wt = sb.tile([P, 2, DIM], FP8, tag="w8s", bufs=10)
                nc.sync.dma_start(wt[:, :, :], c2_d[q])
                c2ts.append(wt)
            pcs = [psp.tile([P, N], F32, tag="acc", bufs=4, name=f"pcs{i}") for i in range(DT)]
            for q in range(CT // 2):
                rhs = hc8s[q // 2][:, (2 * q) % 4:(2 * q) % 4 + 2, :]
                for mt in range(DT):
                    nc.tensor.matmul(pcs[mt][:, :],
                                     c2ts[q][:, :, mt * P:(mt + 1) * P],
                                     rhs,
                                     start=(q == 0), stop=(q == CT // 2 - 1),
                                     perf_mode=DR)
            x3 = []
            for mt in range(DT):
                tb = sb.tile([P, N], F32, tag="lnt", bufs=2)
                nc.scalar.activation(tb[:, :], pcs[mt][:, :], AF.Identity,
                                     bias=c2bt[:, mt:mt + 1], scale=S1INV)
                t = sb.tile([P, N], F32R, tag="x", bufs=6)
                nc.vector.tensor_add(t[:, :], tb[:, :], x2[mt][:, :])
                x3.append(t)

            # ================= ff2 =================
            x4 = ff_block(x3, w3_d, b3t, w4_d, b4t)

            # ================= post-LN =================
            r_b, c_b = layer_norm_rc(x4)
            for mt in range(DT):
                t = sb.tile([P, N], F32, tag="lnt", bufs=2)
                nc.vector.tensor_mul(t[:, :], x4[mt][:, :], r_b[:, :])
                t2 = sb.tile([P, N], F32, tag="lnt2", bufs=1)
                nc.vector.tensor_add(t2[:, :], t[:, :], c_b[:, :])
                ot = sb.tile([P, N], F32, tag="outt", bufs=2)
                nc.vector.tensor_scalar(out=ot[:, :], in0=t2[:, :],
                                        scalar1=pngt[:, mt:mt + 1],
                                        scalar2=pnbt[:, mt:mt + 1],
                                        op0=AL.mult, op1=AL.add)
                eng = (nc.scalar, nc.sync, nc.gpsimd, nc.scalar)[mt]
                eng.dma_start(outT_d[mt * P:(mt + 1) * P, :], ot[:, :])

    if split_waits:
        _split_matmul_waits(nc, mybir)
    return nc


def _split_matmul_waits(nc, mybir):
    """This walrus build rejects engine instructions carrying more than one
    sync wait; hoist the extras onto EventSemaphore instructions on the same
    engine queue right before the instruction."""
    fn = nc.m.functions[0]
    ctr = 0
    for blk in fn.blocks:
        out = []
        changed = False
        for ins in blk.instructions:
            si = ins.sync_info
            if (si is not None and si.on_wait and len(si.on_wait) > 1
                    and not isinstance(ins, (mybir.InstEventSemaphore,
                                             mybir.InstNoOp))):
                waits = list(si.on_wait)
                for w in waits[:-1]:
                    ev = mybir.InstNoOp(
                        name=f"EVW-{ctr}", ins=[], outs=[],
                        sync_info=mybir.SyncInfo(on_wait=[w], on_update=[]))
                    ev.engine = ins.engine
                    ctr += 1
                    out.append(ev)
                ins.sync_info = mybir.SyncInfo(
                    on_wait=[waits[-1]], on_update=list(si.on_update or []))
                changed = True
            out.append(ins)
        if changed:
            blk.instructions = out


def prep_inputs(inputs):
    """Host-side preprocessing: fold LN affines / scales / biases into weights."""
    import ml_dtypes

    f = np.float32
    f8 = ml_dtypes.float8_e4m3
    ii = {k: np.asarray(v, dtype=f) for k, v in inputs.items()}

    def colmaj(b, nb):
        return np.ascontiguousarray(b.astype(f).reshape(nb, P).T)

    def pack_pairs(w, spow):
        """[K, M] weight -> [K//256, P, 2, M] fp8, scaled by 2^spow."""
        K, M = w.shape
        t = (w * (2.0 ** spow)).reshape(K // (2 * P), 2, P, M).transpose(0, 2, 1, 3)
        return np.ascontiguousarray(t.astype(f8))

    g1, be1 = ii["ff1_ln_g"], ii["ff1_ln_b"]
    w1 = pack_pairs(g1[:, None] * ii["ff1_w1"], S1POW)
    b1 = colmaj(be1 @ ii["ff1_w1"] + ii["ff1_b1"], FT)
    w2 = pack_pairs(0.5 * ii["ff1_w2"], S2POW)
    b2 = colmaj(0.5 * ii["ff1_b2"], DT)

    ag, ab = ii["attn_ln_g"], ii["attn_ln_b"]
    sc = DH ** -0.5
    wq = np.ascontiguousarray(ag[:, None] * ii["wq"] * sc)
    bq = colmaj((ab @ ii["wq"] + ii["bq"]) * sc, DT)
    wkv, bkv = ii["wkv"], ii["bkv"]
    wk = np.ascontiguousarray(ag[:, None] * wkv[:, :INNER])
    bk = colmaj(ab @ wkv[:, :INNER] + bkv[:INNER], DT)
    wv = np.ascontiguousarray(ag[:, None] * wkv[:, INNER:])
    bv = np.ascontiguousarray(np.broadcast_to(
        ab @ wkv[:, INNER:] + bkv[INNER:], (P, INNER)))
    wo = pack_pairs(ii["wo"], S1POW)
    bo = colmaj(ii["bo"], DT)
    # relT rows: head feature d lives at partition (h%2)*64 + d -> duplicate rows
    rT = ii["rel_emb"].T[:, ::-1]  # [64, 1025] column-reversed
    relT = np.ascontiguousarray(np.concatenate([rT, rT], axis=0))  # [128, 1025]

    # conv1 in natural [a(0:CIN) | g(CIN:2CIN)] column order
    c1 = pack_pairs(ii["conv1_w"], S1POW)
    c1b = ii["conv1_b"]
    c1a = colmaj(c1b[:CIN], CT)
    c1g = colmaj(c1b[CIN:], CT)
    # depthwise taps as diagonal pair blocks [CT, P, 32, P] (tap 31 zero)
    KP = KW + 1
    dwd = np.zeros((CT, P, KP, P), dtype=f)
    wr = ii["dw_w"].reshape(CT, P, KW) * (2.0 ** SDPOW)
    pp = np.arange(P)
    for ct in range(CT):
        for k in range(KW):
            dwd[ct, pp, k, pp] = wr[ct, :, k]
    dwd = np.ascontiguousarray(dwd.astype(f8))
    inv = 1.0 / np.sqrt(ii["bn_var"] + EPS)
    s = inv * ii["bn_g"]
    t = ii["bn_b"] - ii["bn_mean"] * s
    bns = colmaj(s * (2.0 ** -SDPOW), CT)
    bnt = colmaj(t + s * ii["dw_b"], CT)
    c2 = pack_pairs(ii["conv2_w"], S1POW)
    c2b = colmaj(ii["conv2_b"], DT)

    g3, be3 = ii["ff2_ln_g"], ii["ff2_ln_b"]
    w3 = pack_pairs(g3[:, None] * ii["ff2_w1"], S1POW)
    b3 = colmaj(be3 @ ii["ff2_w1"] + ii["ff2_b1"], FT)
    w4 = pack_pairs(0.5 * ii["ff2_w2"], S2POW)
    b4 = colmaj(0.5 * ii["ff2_b2"], DT)

    png = colmaj(ii["pn_g"], DT)
    pnb = colmaj(ii["pn_b"], DT)

    # pack the small [P, k] constants into one tensor; column map mirrors build()
    pk = np.concatenate([b1, b2, bq, bk, bo, c1a, c1g, bns, bnt, c2b,
                         b3, b4, png, pnb], axis=1)
    assert pk.shape == (P, 96), pk.shape
    shared = dict(w1p=w1, w2p=w2, wq=wq, wk=wk,
                  wv=wv, bvb=bv, wop=wo, relT=relT, c1p=c1,
                  dwp=dwd, c2p=c2,
                  w3p=w3, w4p=w4,
                  cstpack=np.ascontiguousarray(pk),
                  antid=np.ascontiguousarray(np.eye(P, dtype=f)),
                  onesf=np.ones((P, P), dtype=f),
                  onesm=np.full((P, P), 1.0 / DIM, dtype=f))
    x = ii["x"]
    in_maps = []
    for b in range(NCORES):
        m = dict(shared)
        m["xT"] = np.ascontiguousarray(x[b].T)
        in_maps.append(m)
    return in_maps


_BUILT = None


def run(inputs, trace=False):
    global _BUILT
    from concourse import bass_utils

    in_maps = prep_inputs(inputs)
    if _BUILT is None:
        _BUILT = build()
    res = bass_utils.run_bass_kernel_spmd(
        _BUILT, in_maps, core_ids=list(range(NCORES)), trace=trace)
    out = np.stack([np.asarray(r["outT"]).T for r in res.results])
    return np.ascontiguousarray(out.astype(np.float32)), res


def kernel(**inputs):
    out, _ = run(inputs, trace=False)
    return out

